# revision 8
# baseline (speedup 1.0000x reference)
"""Trainium2 Bass kernel for nn_CoOccurrenceMatrixFast.

GLCM (256x256-bin co-occurrence histograms) via one-hot matmuls on the PE
(exact integer counts accumulated in fp32 PSUM), followed by the 6-conv +
3-fc CNN in bf16 with fp32 accumulation.  Pure data parallel: batch 32
sharded as 4 images per NeuronCore across 8 cores.

kernel(**inputs) takes the full unsharded inputs, returns the full [32, 1].
"""

import numpy as np
import ml_dtypes

N_CORES = 8
N_IMG = 4          # images per core
N_CH = 3
NB = 256
MEAN = [0.485, 0.456, 0.406]
STD = [0.229, 0.224, 0.225]
PAD_SENTINEL = 384.0   # never equals -q (q in 0..255); exact in bf16

IMC_R, IMC_C = 258, 260        # conv1 im2col dram buffer [27, 258, 260]
C1O_R, C1O_C = 260, 264        # conv1 output dram buffer [32, 260, 264]

W_SL = 6                       # one-hot window slots
N_SLOT = 513

_BUILD_CACHE = {}


def _pack_weights(inp):
    f32 = np.float32
    bf16 = ml_dtypes.bfloat16
    w1, w2, w3, w4, w5, w6 = (np.asarray(inp[k], f32) for k in
                              ("w1", "w2", "w3", "w4", "w5", "w6"))
    out = {}

    # conv1: lhsT [27, 32];  K row t=(dy*3+dx)*3+ch
    w1p = np.zeros((27, 32), f32)
    for dy in range(3):
        for dx in range(3):
            for ch in range(3):
                w1p[(dy * 3 + dx) * 3 + ch, :] = w1[:, ch, dy, dx]
    out["w1p"] = w1p.astype(bf16)

    # conv2: [8, 2, 128K, 128M]; K=(s*32+ic); M=(oc*4+g)
    w2p = np.zeros((8, 2, 128, 128), f32)
    for r in range(8):
        for g in range(4):
            dy = r - g
            if not (0 <= dy < 5):
                continue
            for s in range(4):
                w2p[r, 0, s * 32:(s + 1) * 32, g::4] = w2[:, :, dy, s].T
            w2p[r, 1, 0:32, g::4] = w2[:, :, dy, 4].T
    out["w2p"] = w2p.astype(bf16)

    # conv3: [3, 128K, 64M]; K=(s*32+ic), s in 0..2
    w3p = np.zeros((3, 128, 64), f32)
    for dy in range(3):
        for s in range(3):
            w3p[dy, s * 32:(s + 1) * 32, :] = w3[:, :, dy, s].T
    out["w3p"] = w3p.astype(bf16)

    # conv4: [6, 3, 128K, 128M]; K=(s*64+ic); M=(oc*2+g)
    w4p = np.zeros((6, 3, 128, 128), f32)
    for r in range(6):
        for g in range(2):
            dy = r - g
            if not (0 <= dy < 5):
                continue
            for s in range(2):
                w4p[r, 0, s * 64:(s + 1) * 64, g::2] = w4[:, :, dy, s].T
                w4p[r, 1, s * 64:(s + 1) * 64, g::2] = w4[:, :, dy, 2 + s].T
            w4p[r, 2, 0:64, g::2] = w4[:, :, dy, 4].T
    out["w4p"] = w4p.astype(bf16)

    # conv5: [3, 2, 128K, 128M]; K=(s*64+ic)
    w5p = np.zeros((3, 2, 128, 128), f32)
    for dy in range(3):
        for s in range(2):
            w5p[dy, 0, s * 64:(s + 1) * 64, :] = w5[:, :, dy, s].T
        w5p[dy, 1, 0:64, :] = w5[:, :, dy, 2].T
    out["w5p"] = w5p.astype(bf16)

    # conv6: [5, 5, 128K(ic), 128M(oc)]
    out["w6p"] = np.ascontiguousarray(w6.transpose(2, 3, 1, 0)).astype(bf16)

    out["b1p"] = np.asarray(inp["b1"], f32).reshape(32, 1)
    out["b2p"] = np.repeat(np.asarray(inp["b2"], f32), 4).reshape(128, 1)
    out["b3p"] = np.asarray(inp["b3"], f32).reshape(64, 1)
    out["b4p"] = np.repeat(np.asarray(inp["b4"], f32), 2).reshape(128, 1)
    out["b5p"] = np.asarray(inp["b5"], f32).reshape(128, 1)
    out["b6p"] = np.asarray(inp["b6"], f32).reshape(128, 1)

    fw1 = np.asarray(inp["fw1"], f32).reshape(256, 128, 1024)     # [o, ch, s]
    out["fw1p"] = np.ascontiguousarray(fw1.transpose(2, 1, 0)).astype(bf16)
    fw2 = np.asarray(inp["fw2"], f32)
    out["fw2p"] = np.ascontiguousarray(fw2.T.reshape(2, 128, 256)).astype(bf16)
    fw3 = np.asarray(inp["fw3"], f32)
    out["fw3p"] = np.ascontiguousarray(fw3.T.reshape(2, 128, 1)).astype(bf16)

    out["fb1bc"] = np.tile(np.asarray(inp["fb1"], f32).reshape(1, 256), (4, 1))
    out["fb2bc"] = np.tile(np.asarray(inp["fb2"], f32).reshape(1, 256), (4, 1))
    out["fb3bc"] = np.tile(np.asarray(inp["fb3"], f32).reshape(1, 1), (4, 1))

    qc = np.zeros((128, 6), f32)
    for c in range(3):
        qc[:, c] = MEAN[c]
        qc[:, 3 + c] = STD[c]
    out["qconst"] = qc

    out["negio4"] = np.tile((-np.arange(256, dtype=f32)).reshape(1, 256, 1),
                            (128, 1, W_SL)).astype(bf16)
    out["id128"] = np.eye(128, dtype=f32)
    out["id4"] = np.eye(4, dtype=f32)
    return out


def _build(debug=False, n_img=N_IMG, n_ch=N_CH):
    import concourse.bass as bass
    import concourse.tile as tile
    import concourse.mybir as mybir
    from concourse import bacc

    dt = mybir.dt
    f32, bf16 = dt.float32, dt.bfloat16

    nc = bacc.Bacc("TRN2", target_bir_lowering=False, debug=False,
                   num_devices=N_CORES)

    env = {}

    def din(name, shape, dtype):
        ap = nc.dram_tensor(name, shape, dtype, kind="ExternalInput").ap()
        env[name] = ap
        return ap

    din("x4", [n_img, N_CH, 256, 256], f32)
    din("w1p", [27, 32], bf16)
    din("w2p", [8, 2, 128, 128], bf16)
    din("w3p", [3, 128, 64], bf16)
    din("w4p", [6, 3, 128, 128], bf16)
    din("w5p", [3, 2, 128, 128], bf16)
    din("w6p", [5, 5, 128, 128], bf16)
    for nm, p in [("b1p", 32), ("b2p", 128), ("b3p", 64), ("b4p", 128),
                  ("b5p", 128), ("b6p", 128)]:
        din(nm, [p, 1], f32)
    din("fw1p", [1024, 128, 256], bf16)
    din("fw2p", [2, 128, 256], bf16)
    din("fw3p", [2, 128, 1], bf16)
    din("fb1bc", [4, 256], f32)
    din("fb2bc", [4, 256], f32)
    din("fb3bc", [4, 1], f32)
    din("negio4", [128, 256, W_SL], bf16)
    din("qconst", [128, 6], f32)
    din("id128", [128, 128], f32)
    din("id4", [4, 4], f32)

    env["out4"] = nc.dram_tensor("out4", [n_img, 1], f32,
                                 kind="ExternalOutput").ap()
    dbg = {}
    if debug:
        def dout(name, shape, dtype=bf16):
            dbg[name] = nc.dram_tensor("dbg_" + name, shape, dtype,
                                       kind="ExternalOutput").ap()
        dout("co", [n_img * n_ch, 128, 512])
        dout("pf2", [32, 128, 128])
        dout("pf3", [64, 128, 128])
        dout("pf4", [64, 64, 64])
        dout("c6in", [128, 68, 72])
        dout("h", [128, n_img, 1024])
        dout("h1", [4, 256], f32)
    env["dbg"] = dbg

    with tile.TileContext(nc) as tc:
        _emit(nc, tc, bass, mybir, env, debug, n_img, n_ch)
    nc.compile()
    return nc


def _emit(nc, tc, bass, mybir, env, debug, n_img, n_ch):
    from contextlib import ExitStack
    dt = mybir.dt
    f32, bf16 = dt.float32, dt.bfloat16
    AF = mybir.ActivationFunctionType
    OP = mybir.AluOpType
    x4 = env["x4"]
    out4 = env["out4"]
    dbg = env["dbg"]

    def rawap(base, extra_off, dims):
        return bass.AP(tensor=base.tensor, offset=base.offset + extra_off,
                       ap=dims)

    ctx = ExitStack()
    singles = ctx.enter_context(tc.tile_pool(name="singles", bufs=1))
    dram = ctx.enter_context(tc.tile_pool(name="dram", bufs=1, space="DRAM"))
    psg = ctx.enter_context(tc.tile_pool(name="psg", bufs=1, space="PSUM"))
    psc = ctx.enter_context(tc.tile_pool(name="psc", bufs=2, space="PSUM"))
    gl = ctx.enter_context(tc.tile_pool(name="gl", bufs=2))
    ohp = ctx.enter_context(tc.tile_pool(name="ohp", bufs=2))
    cv = ctx.enter_context(tc.tile_pool(name="cv", bufs=2))
    big = ctx.enter_context(tc.tile_pool(name="big", bufs=1))

    # ---------------- constants to SBUF ----------------
    def load_const(name, shape, dtype, tag=None):
        t = singles.tile(shape, dtype, tag=tag or name)
        nc.sync.dma_start(out=t[:], in_=env[name])
        return t

    def load_w(dname, ntile_shape, dtype):
        # dram [T..., 128K, M] -> sbuf [128K, T..., M]
        sb = singles.tile(ntile_shape, dtype, tag=dname + "sb")
        d = env[dname]
        K = d.ap[-2][1]
        M = d.ap[-1][1]
        nt = 1
        for s, c in d.ap[:-2]:
            nt *= c
        in_dims = [[d.ap[-2][0], K], [K * M, nt], [1, M]]
        nc.sync.dma_start(out=sb[:], in_=rawap(d, 0, in_dims))
        return sb

    w1sb = load_const("w1p", [27, 32], bf16)
    w2sb = load_w("w2p", [128, 8, 2, 128], bf16)
    w3sb = load_w("w3p", [128, 3, 64], bf16)
    w4sb = load_w("w4p", [128, 6, 3, 128], bf16)
    w5sb = load_w("w5p", [128, 3, 2, 128], bf16)
    w6sb = load_w("w6p", [128, 5, 5, 128], bf16)
    fw2sb = load_w("fw2p", [128, 2, 256], bf16)
    fw3sb = load_w("fw3p", [128, 2, 1], bf16)

    b1sb = load_const("b1p", [32, 1], f32)
    b2sb = load_const("b2p", [128, 1], f32)
    b3sb = load_const("b3p", [64, 1], f32)
    b4sb = load_const("b4p", [128, 1], f32)
    b5sb = load_const("b5p", [128, 1], f32)
    b6sb = load_const("b6p", [128, 1], f32)
    fb1sb = load_const("fb1bc", [4, 256], f32)
    fb2sb = load_const("fb2bc", [4, 256], f32)
    fb3sb = load_const("fb3bc", [4, 1], f32)
    negio4 = load_const("negio4", [128, 256, W_SL], bf16)
    qcsb = load_const("qconst", [128, 6], f32)
    id128 = load_const("id128", [128, 128], f32)
    id4 = load_const("id4", [4, 4], f32)

    # ---------------- DRAM scratch (zero-init once) ----------------
    imc = [dram.tile([27, IMC_R, IMC_C], bf16, tag=f"imc{i}", name=f"imc{i}")
           for i in range(2)]
    c1o = [dram.tile([32, C1O_R, C1O_C], bf16, tag=f"c1o{i}", name=f"c1o{i}")
           for i in range(2)]

    ZW = 512
    zt = singles.tile([128, ZW], bf16, tag="zeros")
    nc.vector.memset(zt[:], 0.0)
    ZN = 128 * ZW
    for buf, total in [(imc[0][:], 27 * IMC_R * IMC_C),
                       (imc[1][:], 27 * IMC_R * IMC_C),
                       (c1o[0][:], 32 * C1O_R * C1O_C),
                       (c1o[1][:], 32 * C1O_R * C1O_C)]:
        off = 0
        while off < total:
            n = min(ZN, total - off)
            p = n // ZW
            if p >= 1:
                nc.gpsimd.dma_start(
                    out=rawap(buf, off, [[ZW, p], [1, ZW]]),
                    in_=zt[:p, :ZW])
                off += p * ZW
            else:
                nc.gpsimd.dma_start(out=rawap(buf, off, [[n, 1], [1, n]]),
                                    in_=zt[:1, :n])
                off += n

    h_sb = singles.tile([128, n_img, 1024], bf16, tag="h")
    NI = n_img
    pfc1 = psg.tile([NI, 256], f32, tag="pfc1")

    # ================= per image =================
    for img in range(n_img):
        imcb = imc[img % 2]
        c1ob = c1o[img % 2]

        # ---------- GLCM for the channels ----------
        for ch in range(n_ch):
            xbase = x4[img, ch]    # [256, 256] dram ap

            xin = gl.tile([128, 2, 256], f32, tag="xin")
            nc.sync.dma_start(out=xin[:], in_=rawap(
                xbase, 0, [[256, 128], [128 * 256, 2], [1, 256]]))
            xin2 = gl.tile([128, 2, 256], f32, tag="xin2")
            nc.sync.dma_start(out=xin2[:, 0, :], in_=rawap(
                xbase, 256, [[256, 128], [1, 256]]))
            nc.vector.memset(xin2[:, 1, :], 0.0)
            nc.sync.dma_start(out=xin2[:127, 1, :], in_=rawap(
                xbase, 129 * 256, [[256, 127], [1, 256]]))

            # quantize: u = min(relu(std*x+mean), 1)*255; store -floor(u)
            qv = gl.tile([128, 2, 520], bf16, tag="qv")
            qv0 = qv[:]
            nc.vector.memset(qv[:, 1, :], PAD_SENTINEL)
            nc.vector.memset(qv[:, 0, 512:513], PAD_SENTINEL)
            for (src, dst_off, blk_parts) in (
                    (xin, 0, (128, 128)), (xin2, 520 + 1, (128, 127))):
                r = gl.tile([128, 2, 256], f32, tag="qr")
                nc.scalar.activation(out=r[:], in_=src[:], func=AF.Relu,
                                     bias=qcsb[:, ch:ch + 1],
                                     scale=qcsb[:, 3 + ch:3 + ch + 1])
                u = gl.tile([128, 2, 256], f32, tag="qu")
                nc.vector.tensor_scalar(out=u[:], in0=r[:], scalar1=1.0,
                                        scalar2=float(NB - 1), op0=OP.min,
                                        op1=OP.mult)
                # HW f32->int cast rounds to nearest; shift so that
                # round(u - 0.5 + eps) == floor(u) incl. integer ties.
                u2 = gl.tile([128, 2, 256], f32, tag="qu2")
                nc.vector.tensor_scalar(out=u2[:], in0=u[:], scalar1=0.5,
                                        scalar2=float(2.0 ** -15),
                                        op0=OP.subtract, op1=OP.add)
                qi = gl.tile([128, 2, 256], dt.int32, tag="qi")
                nc.vector.tensor_copy(out=qi[:], in_=u2[:])
                for blk in range(2):
                    np_ = blk_parts[blk]
                    dst = rawap(qv0, dst_off + 256 * blk,
                                [[qv0.ap[0][0], np_], [1, 256]])
                    qin = rawap(qi[:], 256 * blk,
                                [[qi[:].ap[0][0], np_], [1, 256]])
                    nc.vector.tensor_scalar(out=dst, in0=qin, scalar1=-1.0,
                                            scalar2=None, op0=OP.mult)

            # G psum: [128, 512] = [G_h | G_v] per left-half
            G = [psg.tile([128, 512], f32, tag=f"G{h}", name=f"G{h}")
                 for h in range(2)]

            n_win = (N_SLOT + W_SL - 1) // W_SL
            ohw = [None] * n_win
            first = [True, True]

            # one-hot layout [128, blk, value, slot]: every DVE operand has
            # inner stride 1 / 2-byte dtype, so is_equal runs in 2x perf
            # mode.  Matmuls read the slot dim at stride W_SL.
            def build_window(w):
                nslots = min(W_SL, N_SLOT - w * W_SL)
                t = ohp.tile([128, 2, 256, W_SL], bf16, tag="ohw")
                in0 = rawap(qv0, w * W_SL,
                            [qv0.ap[0], [520, 2], [0, 256], [1, nslots]])
                in1 = rawap(negio4[:], 0,
                            [negio4[:].ap[0], [0, 2], [W_SL, 256],
                             [1, nslots]])
                nc.vector.tensor_tensor(out=t[:, :, :, 0:nslots], in0=in0,
                                        in1=in1, op=OP.is_equal)
                ohw[w] = t

            def chunk_mm(t):
                w1_, s1 = divmod(t, W_SL)
                w2_, s2 = divmod(t + 1, W_SL)
                stop = t == 511
                o1 = ohw[w1_][:]
                o2 = ohw[w2_][:]
                if t == 255:
                    rhs = rawap(o2, 256 * W_SL + s2, [o2.ap[0], [W_SL, 256]])
                    for h in range(2):
                        lhsT = rawap(o1, h * 128 * W_SL + s1,
                                     [o1.ap[0], [W_SL, 128]])
                        nc.tensor.matmul(G[h][:, 256:512], lhsT, rhs,
                                         start=False, stop=False)
                    return
                rhs = rawap(o2, s2,
                            [o2.ap[0], [256 * W_SL, 2], [W_SL, 256]])
                for h in range(2):
                    lhsT = rawap(o1, h * 128 * W_SL + s1,
                                 [o1.ap[0], [W_SL, 128]])
                    nc.tensor.matmul(G[h][:], lhsT, rhs,
                                     start=first[h], stop=stop)
                    first[h] = False

            for w in range(n_win):
                build_window(w)
                lo = max(0, w * W_SL - 1)
                hi = min(512, w * W_SL + W_SL - 1)
                for t in range(lo, hi):
                    chunk_mm(t)

            # S = G_h + G_v ; co = S + S^T
            s_half = []
            for h in range(2):
                tmp = gl.tile([128, 256], f32, tag="stmp")
                nc.scalar.activation(out=tmp[:], in_=G[h][:, 0:256],
                                     func=AF.Copy)
                s = gl.tile([128, 256], f32, tag=f"s{h}")
                nc.vector.tensor_tensor(out=s[:], in0=G[h][:, 256:512],
                                        in1=tmp[:], op=OP.add)
                s_half.append(s)
            co_t = gl.tile([128, 2, 256], bf16, tag="co")
            for h in range(2):
                tp = psc.tile([128, 256], f32, tag="pc")
                for j in range(2):
                    nc.tensor.matmul(tp[:, j * 128:(j + 1) * 128],
                                     s_half[j][:, h * 128:(h + 1) * 128],
                                     id128[:], start=True, stop=True)
                nc.vector.tensor_tensor(out=co_t[:, h, :], in0=tp[:],
                                        in1=s_half[h][:], op=OP.add)
            if debug:
                nc.sync.dma_start(
                    out=dbg["co"][img * n_ch + ch],
                    in_=co_t[:].rearrange("p a b -> p (a b)"))

            # 9 tap-shifted replicas into the conv1 im2col buffer
            P_IMC = IMC_R * IMC_C
            for dy in range(3):
                for dx in range(3):
                    tpart = (dy * 3 + dx) * 3 + ch
                    off = tpart * P_IMC + (2 - dy) * IMC_C + (2 - dx)
                    nc.gpsimd.dma_start(
                        out=rawap(imcb[:], off,
                                  [[IMC_C, 128], [128 * IMC_C, 2], [1, 256]]),
                        in_=co_t[:])

        if n_ch < 3:
            continue   # debug mode without convs

        # ---------- conv1 (27 -> 32, relu) ----------
        for band in range(16):
            y0 = band * 16
            bt1 = cv.tile([27, 16, 260], bf16, tag="cvA")
            nc.sync.dma_start(out=bt1[:], in_=imcb[:, y0 + 1:y0 + 17, :])
            st1 = cv.tile([32, 16, 256], bf16, tag="cvB")
            for ci in range(8):
                p1 = psc.tile([32, 512], f32, tag="pc")
                nc.tensor.matmul(p1[:], w1sb[:],
                                 bt1[:, 2 * ci:2 * ci + 2, 1:257],
                                 start=True, stop=True)
                nc.scalar.activation(out=st1[:, 2 * ci:2 * ci + 2, :],
                                     in_=p1[:], func=AF.Relu, bias=b1sb[:])
            nc.gpsimd.dma_start(out=c1ob[:, 2 + y0:2 + y0 + 16, 2:258],
                                in_=st1[:])
        # ---------- conv2 (32 -> 32, 5x5, pool) ----------
        pf2 = big.tile([32, 128, 128], bf16, tag="pf")
        for band in range(32):
            y0 = band * 8
            bt2 = cv.tile([128, 12, 260], bf16, tag="cvA")
            for s in range(4):
                nc.sync.dma_start(out=bt2[s * 32:(s + 1) * 32, :, :],
                                  in_=rawap(
                    c1ob[:], y0 * C1O_C + s,
                    [[C1O_R * C1O_C, 32], [C1O_C, 12], [1, 260]]))
            rft2 = cv.tile([32, 8, 256], bf16, tag="cvB")
            for ci in range(2):
                p2 = psc.tile([128, 256], f32, tag="pc")
                for rr in range(8):
                    for dxg in range(2):
                        nc.tensor.matmul(
                            p2[:], w2sb[:, rr, dxg, :],
                            bt2[:, 4 * ci + rr, 4 * dxg:4 * dxg + 256],
                            start=(rr == 0 and dxg == 0),
                            stop=(rr == 7 and dxg == 1))
                bsb2 = cv.tile([128, 256], bf16, tag="cvC")
                nc.scalar.activation(out=bsb2[:], in_=p2[:],
                                     func=AF.Identity, bias=b2sb[:])
                nc.gpsimd.dma_start(
                    out=rawap(rft2[:], ci * 1024,
                              [rft2[:].ap[0], [256, 4], [1, 256]]),
                    in_=bsb2[:])
            tmp2 = cv.tile([32, 4, 256], bf16, tag="cvD")
            nc.vector.tensor_tensor(
                out=tmp2[:],
                in0=rawap(rft2[:], 0, [rft2[:].ap[0], [512, 4], [1, 256]]),
                in1=rawap(rft2[:], 256, [rft2[:].ap[0], [512, 4], [1, 256]]),
                op=OP.max)
            nc.vector.tensor_tensor(
                out=pf2[:, band * 4:band * 4 + 4, :],
                in0=rawap(tmp2[:], 0, [tmp2[:].ap[0], [256, 4], [2, 128]]),
                in1=rawap(tmp2[:], 1, [tmp2[:].ap[0], [256, 4], [2, 128]]),
                op=OP.max)
        if debug and img == n_img - 1:
            nc.sync.dma_start(out=dbg["pf2"][:], in_=pf2[:])

        # ---------- conv3 input (4-shift replicate, pad 1) ----------
        c3in = big.tile([128, 130, 132], bf16, tag="cio")
        nc.vector.memset(c3in[:], 0.0)
        for s in range(4):
            c0 = max(0, 1 - s)
            cN = min(132, 129 - s)
            nc.gpsimd.dma_start(
                out=c3in[s * 32:(s + 1) * 32, 1:129, c0:cN],
                in_=pf2[:, :, c0 - 1 + s:cN - 1 + s])

        # ---------- conv3 (32 -> 64, 3x3, relu) ----------
        pf3 = big.tile([64, 128, 128], bf16, tag="pf")
        for ci in range(32):
            y = ci * 4
            p3 = psc.tile([64, 512], f32, tag="pc")
            for dy in range(3):
                nc.tensor.matmul(p3[:], w3sb[:, dy, :],
                                 c3in[:, y + dy:y + dy + 4, 0:128],
                                 start=(dy == 0), stop=(dy == 2))
            nc.scalar.activation(out=pf3[:, y:y + 4, :], in_=p3[:],
                                 func=AF.Relu, bias=b3sb[:])
        if debug and img == n_img - 1:
            nc.sync.dma_start(out=dbg["pf3"][:], in_=pf3[:])

        # ---------- conv4 input (2-shift replicate, pad 2) ----------
        c4in = big.tile([128, 132, 132], bf16, tag="cio")
        nc.vector.memset(c4in[:], 0.0)
        for s in range(2):
            c0 = max(0, 2 - s)
            cN = min(132, 130 - s)
            nc.gpsimd.dma_start(
                out=c4in[s * 64:(s + 1) * 64, 2:130, c0:cN],
                in_=pf3[:, :, c0 - 2 + s:cN - 2 + s])

        # ---------- conv4 (64 -> 64, 5x5, pool) ----------
        pf4 = big.tile([64, 64, 64], bf16, tag="pf")
        for grp in range(16):     # 16 groups of 4 chunks (2 rows each)
            rft4 = cv.tile([64, 8, 128], bf16, tag="cvB")
            for cj in range(4):
                ci = grp * 4 + cj
                y = ci * 2
                p4 = psc.tile([128, 128], f32, tag="pc")
                for rr in range(6):
                    for dxg in range(3):
                        nc.tensor.matmul(
                            p4[:], w4sb[:, rr, dxg, :],
                            c4in[:, y + rr, 2 * dxg:2 * dxg + 128],
                            start=(rr == 0 and dxg == 0),
                            stop=(rr == 5 and dxg == 2))
                bsb4 = cv.tile([128, 128], bf16, tag="cvC")
                nc.scalar.activation(out=bsb4[:], in_=p4[:],
                                     func=AF.Identity, bias=b4sb[:])
                nc.gpsimd.dma_start(
                    out=rawap(rft4[:], cj * 256,
                              [rft4[:].ap[0], [128, 2], [1, 128]]),
                    in_=bsb4[:])
            tmp4 = cv.tile([64, 4, 128], bf16, tag="cvD")
            nc.vector.tensor_tensor(
                out=tmp4[:],
                in0=rawap(rft4[:], 0, [rft4[:].ap[0], [256, 4], [1, 128]]),
                in1=rawap(rft4[:], 128, [rft4[:].ap[0], [256, 4], [1, 128]]),
                op=OP.max)
            nc.vector.tensor_tensor(
                out=pf4[:, grp * 4:grp * 4 + 4, :],
                in0=rawap(tmp4[:], 0, [tmp4[:].ap[0], [128, 4], [2, 64]]),
                in1=rawap(tmp4[:], 1, [tmp4[:].ap[0], [128, 4], [2, 64]]),
                op=OP.max)
        if debug and img == n_img - 1:
            nc.sync.dma_start(out=dbg["pf4"][:], in_=pf4[:])

        # ---------- conv5 input (2-shift replicate, pad 1) ----------
        c5in = big.tile([128, 66, 68], bf16, tag="cio")
        nc.vector.memset(c5in[:], 0.0)
        for s in range(2):
            c0 = max(0, 1 - s)
            cN = min(68, 65 - s)
            nc.gpsimd.dma_start(
                out=c5in[s * 64:(s + 1) * 64, 1:65, c0:cN],
                in_=pf4[:, :, c0 - 1 + s:cN - 1 + s])

        # ---------- conv5 (64 -> 128, 3x3, relu) ----------
        c6in = big.tile([128, 68, 72], bf16, tag="c6in")
        nc.vector.memset(c6in[:], 0.0)
        for ci in range(16):
            y = ci * 4
            p5 = psc.tile([128, 256], f32, tag="pc")
            for dy in range(3):
                for dxg in range(2):
                    nc.tensor.matmul(
                        p5[:], w5sb[:, dy, dxg, :],
                        rawap(c5in[:], (y + dy) * 68 + 2 * dxg,
                              [c5in[:].ap[0], [68, 4], [1, 64]]),
                        start=(dy == 0 and dxg == 0),
                        stop=(dy == 2 and dxg == 1))
            nc.scalar.activation(out=c6in[:, 2 + y:2 + y + 4, 2:66],
                                 in_=p5[:], func=AF.Relu, bias=b5sb[:])
        if debug and img == n_img - 1:
            nc.sync.dma_start(out=dbg["c6in"][:], in_=c6in[:])

        # ---------- conv6 (128 -> 128, 5x5, pool) ----------
        for ci in range(8):
            y = ci * 8
            p6 = psc.tile([128, 512], f32, tag="pc")
            for dy in range(5):
                for dx in range(5):
                    nc.tensor.matmul(
                        p6[:], w6sb[:, dy, dx, :],
                        rawap(c6in[:], (y + dy) * 72 + dx,
                              [c6in[:].ap[0], [72, 8], [1, 64]]),
                        start=(dy == 0 and dx == 0),
                        stop=(dy == 4 and dx == 4))
            sb6 = cv.tile([128, 8, 64], bf16, tag="cvC")
            nc.scalar.activation(out=sb6[:], in_=p6[:], func=AF.Identity,
                                 bias=b6sb[:])
            t6 = cv.tile([128, 4, 64], bf16, tag="cvD")
            nc.vector.tensor_tensor(
                out=t6[:],
                in0=rawap(sb6[:], 0, [sb6[:].ap[0], [128, 4], [1, 64]]),
                in1=rawap(sb6[:], 64, [sb6[:].ap[0], [128, 4], [1, 64]]),
                op=OP.max)
            hout = rawap(h_sb[:], img * 1024 + ci * 128,
                         [h_sb[:].ap[0], [32, 4], [1, 32]])
            nc.vector.tensor_tensor(
                out=hout,
                in0=rawap(t6[:], 0, [t6[:].ap[0], [64, 4], [2, 32]]),
                in1=rawap(t6[:], 1, [t6[:].ap[0], [64, 4], [2, 32]]),
                op=OP.max)

    if n_ch == 3:
        if debug:
            nc.sync.dma_start(out=dbg["h"][:], in_=h_sb[:])
        # ================= fc layers =================
        FW1_BLK = 8
        for kb in range(1024 // FW1_BLK):
            fwt = cv.tile([128, FW1_BLK, 256], bf16, tag="fwt")
            nc.sync.dma_start(
                out=fwt[:],
                in_=rawap(env["fw1p"], kb * FW1_BLK * 128 * 256,
                          [[256, 128], [128 * 256, FW1_BLK], [1, 256]]))
            for j in range(FW1_BLK):
                s = kb * FW1_BLK + j
                nc.tensor.matmul(pfc1[:], h_sb[:, :, s], fwt[:, j, :],
                                 start=(s == 0), stop=(s == 1023))

        h1 = singles.tile([NI, 256], f32, tag="h1")
        nc.vector.tensor_tensor(out=h1[:], in0=pfc1[:], in1=fb1sb[:NI, :],
                                op=OP.add)
        nc.vector.tensor_scalar_max(h1[:], h1[:], 0.0)
        if debug:
            nc.sync.dma_start(out=dbg["h1"][:NI, :], in_=h1[:])

        h1T = singles.tile([128, 2, NI], bf16, tag="h1T")
        for j in range(2):
            ptp = psc.tile([128, NI], f32, tag="pc")
            nc.tensor.matmul(ptp[:], h1[:, j * 128:(j + 1) * 128],
                             id4[:NI, :NI], start=True, stop=True)
            nc.scalar.activation(out=h1T[:, j, :], in_=ptp[:], func=AF.Copy)

        pfc2 = psc.tile([NI, 256], f32, tag="pc")
        for j in range(2):
            nc.tensor.matmul(pfc2[:], h1T[:, j, :], fw2sb[:, j, :],
                             start=(j == 0), stop=(j == 1))
        h2 = singles.tile([NI, 256], f32, tag="h2")
        nc.vector.tensor_tensor(out=h2[:], in0=pfc2[:], in1=fb2sb[:NI, :],
                                op=OP.add)
        nc.vector.tensor_scalar_max(h2[:], h2[:], 0.0)

        h2T = singles.tile([128, 2, NI], bf16, tag="h2T")
        for j in range(2):
            ptp = psc.tile([128, NI], f32, tag="pc")
            nc.tensor.matmul(ptp[:], h2[:, j * 128:(j + 1) * 128],
                             id4[:NI, :NI], start=True, stop=True)
            nc.scalar.activation(out=h2T[:, j, :], in_=ptp[:], func=AF.Copy)

        pfc3 = psc.tile([NI, 1], f32, tag="pc")
        for j in range(2):
            nc.tensor.matmul(pfc3[:], h2T[:, j, :], fw3sb[:, j, :],
                             start=(j == 0), stop=(j == 1))
        osb = singles.tile([NI, 1], f32, tag="osb")
        nc.scalar.activation(out=osb[:], in_=pfc3[:], func=AF.Sigmoid,
                             bias=fb3sb[:NI, :])
        nc.sync.dma_start(out=out4, in_=osb[:])
    else:
        # tiny debug build: just write something to out4
        osb = singles.tile([4, 1], f32, tag="osb")
        nc.vector.memset(osb[:], 0.0)
        nc.sync.dma_start(out=out4, in_=osb[:n_img, :])

    ctx.close()


def kernel(**inputs):
    from concourse.bass_utils import run_bass_kernel_spmd

    inputs = dict(inputs)
    debug = bool(inputs.pop("_debug", False))
    trace = bool(inputs.pop("_trace", False))
    key = ("k", debug)
    if key not in _BUILD_CACHE:
        _BUILD_CACHE[key] = _build(debug=debug)
    nc = _BUILD_CACHE[key]

    packed = _pack_weights(inputs)
    x = np.asarray(inputs["x"], np.float32)
    in_maps = []
    for c in range(N_CORES):
        m = dict(packed)
        m["x4"] = np.ascontiguousarray(x[c * N_IMG:(c + 1) * N_IMG])
        in_maps.append(m)

    res = run_bass_kernel_spmd(nc, in_maps, core_ids=list(range(N_CORES)),
                               trace=trace)
    out = np.concatenate([res.results[c]["out4"] for c in range(N_CORES)],
                         axis=0)
    kernel._last_results = res
    return out



# revision 13
# speedup vs baseline: 2.4359x; 2.4359x over previous
"""Trainium2 Bass kernel for nn_CoOccurrenceMatrixFast.

GLCM (256x256-bin co-occurrence histograms) via one-hot matmuls on the PE
(exact integer counts accumulated in fp32 PSUM), followed by the 6-conv +
3-fc CNN in bf16 with fp32 accumulation.  Pure data parallel: batch 32
sharded as 4 images per NeuronCore across 8 cores.

kernel(**inputs) takes the full unsharded inputs, returns the full [32, 1].
"""

import numpy as np
import ml_dtypes

N_CORES = 8
N_IMG = 4          # images per core
N_CH = 3
NB = 256
MEAN = [0.485, 0.456, 0.406]
STD = [0.229, 0.224, 0.225]
PAD_SENTINEL = 384.0   # never equals -q (q in 0..255); exact in bf16

IMC_R, IMC_C = 258, 260        # conv1 im2col dram buffer [27, 258, 260]
C1O_R, C1O_C = 260, 264        # conv1 output dram buffer [32, 260, 264]

W_SL = 6                       # one-hot window slots
N_SLOT = 513

_BUILD_CACHE = {}


def _pack_weights(inp):
    f32 = np.float32
    bf16 = ml_dtypes.bfloat16
    w1, w2, w3, w4, w5, w6 = (np.asarray(inp[k], f32) for k in
                              ("w1", "w2", "w3", "w4", "w5", "w6"))
    out = {}

    # conv1: lhsT [27, 32];  K row t=(dy*3+dx)*3+ch
    w1p = np.zeros((27, 32), f32)
    for dy in range(3):
        for dx in range(3):
            for ch in range(3):
                w1p[(dy * 3 + dx) * 3 + ch, :] = w1[:, ch, dy, dx]
    out["w1p"] = w1p.astype(bf16)

    # conv2: [8, 2, 128K, 128M]; K=(s*32+ic); M=(oc*4+g)
    w2p = np.zeros((8, 2, 128, 128), f32)
    for r in range(8):
        for g in range(4):
            dy = r - g
            if not (0 <= dy < 5):
                continue
            for s in range(4):
                w2p[r, 0, s * 32:(s + 1) * 32, g::4] = w2[:, :, dy, s].T
            w2p[r, 1, 0:32, g::4] = w2[:, :, dy, 4].T
    out["w2p"] = w2p.astype(bf16)

    # conv3: [3, 128K, 64M]; K=(s*32+ic), s in 0..2
    w3p = np.zeros((3, 128, 64), f32)
    for dy in range(3):
        for s in range(3):
            w3p[dy, s * 32:(s + 1) * 32, :] = w3[:, :, dy, s].T
    out["w3p"] = w3p.astype(bf16)

    # conv4: [6, 3, 128K, 128M]; K=(s*64+ic); M=(oc*2+g)
    w4p = np.zeros((6, 3, 128, 128), f32)
    for r in range(6):
        for g in range(2):
            dy = r - g
            if not (0 <= dy < 5):
                continue
            for s in range(2):
                w4p[r, 0, s * 64:(s + 1) * 64, g::2] = w4[:, :, dy, s].T
                w4p[r, 1, s * 64:(s + 1) * 64, g::2] = w4[:, :, dy, 2 + s].T
            w4p[r, 2, 0:64, g::2] = w4[:, :, dy, 4].T
    out["w4p"] = w4p.astype(bf16)

    # conv5: [3, 2, 128K, 128M]; K=(s*64+ic)
    w5p = np.zeros((3, 2, 128, 128), f32)
    for dy in range(3):
        for s in range(2):
            w5p[dy, 0, s * 64:(s + 1) * 64, :] = w5[:, :, dy, s].T
        w5p[dy, 1, 0:64, :] = w5[:, :, dy, 2].T
    out["w5p"] = w5p.astype(bf16)

    # conv6: [5, 5, 128K(ic), 128M(oc)]
    out["w6p"] = np.ascontiguousarray(w6.transpose(2, 3, 1, 0)).astype(bf16)

    out["b1p"] = np.asarray(inp["b1"], f32).reshape(32, 1)
    out["b2p"] = np.repeat(np.asarray(inp["b2"], f32), 4).reshape(128, 1)
    out["b3p"] = np.asarray(inp["b3"], f32).reshape(64, 1)
    out["b4p"] = np.repeat(np.asarray(inp["b4"], f32), 2).reshape(128, 1)
    out["b5p"] = np.asarray(inp["b5"], f32).reshape(128, 1)
    out["b6p"] = np.asarray(inp["b6"], f32).reshape(128, 1)

    fw1 = np.asarray(inp["fw1"], f32).reshape(256, 128, 1024)     # [o, ch, s]
    out["fw1p"] = np.ascontiguousarray(fw1.transpose(2, 1, 0)).astype(bf16)
    fw2 = np.asarray(inp["fw2"], f32)
    out["fw2p"] = np.ascontiguousarray(fw2.T.reshape(2, 128, 256)).astype(bf16)
    fw3 = np.asarray(inp["fw3"], f32)
    out["fw3p"] = np.ascontiguousarray(fw3.T.reshape(2, 128, 1)).astype(bf16)

    out["fb1bc"] = np.tile(np.asarray(inp["fb1"], f32).reshape(1, 256), (4, 1))
    out["fb2bc"] = np.tile(np.asarray(inp["fb2"], f32).reshape(1, 256), (4, 1))
    out["fb3bc"] = np.tile(np.asarray(inp["fb3"], f32).reshape(1, 1), (4, 1))

    qc = np.zeros((128, 6), f32)
    for c in range(3):
        qc[:, c] = MEAN[c]
        qc[:, 3 + c] = STD[c]
    out["qconst"] = qc

    out["negiota"] = np.tile((-np.arange(256, dtype=f32)).reshape(1, 256),
                             (128, 1)).astype(bf16)
    out["id128"] = np.eye(128, dtype=f32)
    out["id4"] = np.eye(4, dtype=f32)
    return out


def _build(debug=False, n_img=N_IMG, n_ch=N_CH):
    import concourse.bass as bass
    import concourse.tile as tile
    import concourse.mybir as mybir
    from concourse import bacc

    dt = mybir.dt
    f32, bf16 = dt.float32, dt.bfloat16

    nc = bacc.Bacc("TRN2", target_bir_lowering=False, debug=False,
                   num_devices=N_CORES)

    env = {}

    def din(name, shape, dtype):
        ap = nc.dram_tensor(name, shape, dtype, kind="ExternalInput").ap()
        env[name] = ap
        return ap

    din("x4", [n_img, N_CH, 256, 256], f32)
    din("w1p", [27, 32], bf16)
    din("w2p", [8, 2, 128, 128], bf16)
    din("w3p", [3, 128, 64], bf16)
    din("w4p", [6, 3, 128, 128], bf16)
    din("w5p", [3, 2, 128, 128], bf16)
    din("w6p", [5, 5, 128, 128], bf16)
    for nm, p in [("b1p", 32), ("b2p", 128), ("b3p", 64), ("b4p", 128),
                  ("b5p", 128), ("b6p", 128)]:
        din(nm, [p, 1], f32)
    din("fw1p", [1024, 128, 256], bf16)
    din("fw2p", [2, 128, 256], bf16)
    din("fw3p", [2, 128, 1], bf16)
    din("fb1bc", [4, 256], f32)
    din("fb2bc", [4, 256], f32)
    din("fb3bc", [4, 1], f32)
    din("negiota", [128, 256], bf16)
    din("qconst", [128, 6], f32)
    din("id128", [128, 128], f32)
    din("id4", [4, 4], f32)

    env["out4"] = nc.dram_tensor("out4", [n_img, 1], f32,
                                 kind="ExternalOutput").ap()
    dbg = {}
    if debug:
        def dout(name, shape, dtype=bf16):
            dbg[name] = nc.dram_tensor("dbg_" + name, shape, dtype,
                                       kind="ExternalOutput").ap()
        dout("co", [n_img * n_ch, 128, 512])
        dout("pf2", [32, 128, 128])
        dout("pf3", [64, 128, 128])
        dout("pf4", [64, 64, 64])
        dout("c6in", [128, 68, 72])
        dout("h", [128, n_img, 1024])
        dout("h1", [4, 256], f32)
    env["dbg"] = dbg

    with tile.TileContext(nc) as tc:
        _emit(nc, tc, bass, mybir, env, debug, n_img, n_ch)
    nc.compile()
    return nc


def _emit(nc, tc, bass, mybir, env, debug, n_img, n_ch):
    from contextlib import ExitStack
    dt = mybir.dt
    f32, bf16 = dt.float32, dt.bfloat16
    AF = mybir.ActivationFunctionType
    OP = mybir.AluOpType
    x4 = env["x4"]
    out4 = env["out4"]
    dbg = env["dbg"]

    def rawap(base, extra_off, dims):
        return bass.AP(tensor=base.tensor, offset=base.offset + extra_off,
                       ap=dims)

    ctx = ExitStack()
    singles = ctx.enter_context(tc.tile_pool(name="singles", bufs=1))
    dram = ctx.enter_context(tc.tile_pool(name="dram", bufs=1, space="DRAM"))
    psg = ctx.enter_context(tc.tile_pool(name="psg", bufs=1, space="PSUM"))
    psc = ctx.enter_context(tc.tile_pool(name="psc", bufs=2, space="PSUM"))
    gl = ctx.enter_context(tc.tile_pool(name="gl", bufs=2))
    ohp = ctx.enter_context(tc.tile_pool(name="ohp", bufs=2))
    cv = ctx.enter_context(tc.tile_pool(name="cv", bufs=2))
    big = ctx.enter_context(tc.tile_pool(name="big", bufs=1))

    # ---------------- constants to SBUF ----------------
    def load_const(name, shape, dtype, tag=None):
        t = singles.tile(shape, dtype, tag=tag or name)
        nc.sync.dma_start(out=t[:], in_=env[name])
        return t

    def load_w(dname, ntile_shape, dtype):
        # dram [T..., 128K, M] -> sbuf [128K, T..., M]
        sb = singles.tile(ntile_shape, dtype, tag=dname + "sb")
        d = env[dname]
        K = d.ap[-2][1]
        M = d.ap[-1][1]
        nt = 1
        for s, c in d.ap[:-2]:
            nt *= c
        in_dims = [[d.ap[-2][0], K], [K * M, nt], [1, M]]
        nc.sync.dma_start(out=sb[:], in_=rawap(d, 0, in_dims))
        return sb

    w1sb = load_const("w1p", [27, 32], bf16)
    w2sb = load_w("w2p", [128, 8, 2, 128], bf16)
    w3sb = load_w("w3p", [128, 3, 64], bf16)
    w4sb = load_w("w4p", [128, 6, 3, 128], bf16)
    w5sb = load_w("w5p", [128, 3, 2, 128], bf16)
    w6sb = load_w("w6p", [128, 5, 5, 128], bf16)
    fw2sb = load_w("fw2p", [128, 2, 256], bf16)
    fw3sb = load_w("fw3p", [128, 2, 1], bf16)

    b1sb = load_const("b1p", [32, 1], f32)
    b2sb = load_const("b2p", [128, 1], f32)
    b3sb = load_const("b3p", [64, 1], f32)
    b4sb = load_const("b4p", [128, 1], f32)
    b5sb = load_const("b5p", [128, 1], f32)
    b6sb = load_const("b6p", [128, 1], f32)
    fb1sb = load_const("fb1bc", [4, 256], f32)
    fb2sb = load_const("fb2bc", [4, 256], f32)
    fb3sb = load_const("fb3bc", [4, 1], f32)
    negio = load_const("negiota", [128, 256], bf16)
    qcsb = load_const("qconst", [128, 6], f32)
    id128 = load_const("id128", [128, 128], f32)
    id4 = load_const("id4", [4, 4], f32)

    # ---------------- DRAM scratch (zero-init once) ----------------
    imc = [dram.tile([27, IMC_R, IMC_C], bf16, tag=f"imc{i}", name=f"imc{i}")
           for i in range(2)]
    c1o = [dram.tile([32, C1O_R, C1O_C], bf16, tag=f"c1o{i}", name=f"c1o{i}")
           for i in range(2)]

    ZW = 512
    zt = singles.tile([128, ZW], bf16, tag="zeros")
    nc.vector.memset(zt[:], 0.0)
    ZN = 128 * ZW
    for buf, total in [(imc[0][:], 27 * IMC_R * IMC_C),
                       (imc[1][:], 27 * IMC_R * IMC_C),
                       (c1o[0][:], 32 * C1O_R * C1O_C),
                       (c1o[1][:], 32 * C1O_R * C1O_C)]:
        off = 0
        while off < total:
            n = min(ZN, total - off)
            p = n // ZW
            if p >= 1:
                nc.gpsimd.dma_start(
                    out=rawap(buf, off, [[ZW, p], [1, ZW]]),
                    in_=zt[:p, :ZW])
                off += p * ZW
            else:
                nc.gpsimd.dma_start(out=rawap(buf, off, [[n, 1], [1, n]]),
                                    in_=zt[:1, :n])
                off += n

    h_sb = singles.tile([128, n_img, 1024], bf16, tag="h")
    NI = n_img
    pfc1 = psg.tile([NI, 256], f32, tag="pfc1")

    # ================= per image =================
    for img in range(n_img):
        imcb = imc[img % 2]
        c1ob = c1o[img % 2]

        # ---------- GLCM for the channels ----------
        for ch in range(n_ch):
            xbase = x4[img, ch]    # [256, 256] dram ap

            xin = gl.tile([128, 2, 256], f32, tag="xin")
            nc.sync.dma_start(out=xin[:], in_=rawap(
                xbase, 0, [[256, 128], [128 * 256, 2], [1, 256]]))
            xin2 = gl.tile([128, 2, 256], f32, tag="xin2")
            nc.sync.dma_start(out=xin2[:, 0, :], in_=rawap(
                xbase, 256, [[256, 128], [1, 256]]))
            nc.vector.memset(xin2[:, 1, :], 0.0)
            nc.sync.dma_start(out=xin2[:127, 1, :], in_=rawap(
                xbase, 129 * 256, [[256, 127], [1, 256]]))

            # quantize: u = min(relu(std*x+mean), 1)*255; store -floor(u).
            # Each value is stored TWICE ([s,2] pairs) so the one-hot
            # is_equal can read it with a last-dim [1,2] stride-1 AP,
            # qualifying for the DVE 2x perf mode.
            qv = gl.tile([128, 2, 520, 2], bf16, tag="qv")
            qv0 = qv[:]
            nc.vector.memset(qv[:, 1, :, :], PAD_SENTINEL)
            nc.vector.memset(qv[:, 0, 512:513, :], PAD_SENTINEL)
            for (src, dst_off, blk_parts) in (
                    (xin, 0, (128, 128)), (xin2, 520 + 1, (128, 127))):
                r = gl.tile([128, 2, 256], f32, tag="qr")
                nc.scalar.activation(out=r[:], in_=src[:], func=AF.Relu,
                                     bias=qcsb[:, ch:ch + 1],
                                     scale=qcsb[:, 3 + ch:3 + ch + 1])
                u = gl.tile([128, 2, 256], f32, tag="qu")
                nc.vector.tensor_scalar(out=u[:], in0=r[:], scalar1=1.0,
                                        scalar2=float(NB - 1), op0=OP.min,
                                        op1=OP.mult)
                # HW f32->int cast rounds to nearest; shift so that
                # round(u - 0.5 + eps) == floor(u) incl. integer ties.
                u2 = gl.tile([128, 2, 256], f32, tag="qu2")
                nc.vector.tensor_scalar(out=u2[:], in0=u[:], scalar1=0.5,
                                        scalar2=float(2.0 ** -15),
                                        op0=OP.subtract, op1=OP.add)
                qi = gl.tile([128, 2, 256], dt.int32, tag="qi")
                nc.vector.tensor_copy(out=qi[:], in_=u2[:])
                for blk in range(2):
                    np_ = blk_parts[blk]
                    L = dst_off + 256 * blk
                    noff = (L // 520) * 1040 + (L % 520) * 2
                    dst = rawap(qv0, noff,
                                [[qv0.ap[0][0], np_], [2, 256], [1, 2]])
                    qin = rawap(qi[:], 256 * blk,
                                [[qi[:].ap[0][0], np_], [1, 256], [0, 2]])
                    nc.vector.tensor_scalar(out=dst, in0=qin, scalar1=-1.0,
                                            scalar2=None, op0=OP.mult)

            # G psum: [128, 512] = [G_h | G_v] per left-half
            G = [psg.tile([128, 512], f32, tag=f"G{h}", name=f"G{h}")
                 for h in range(2)]

            n_win = (N_SLOT + W_SL - 1) // W_SL
            ohw = [None] * n_win
            first = [True, True]

            # one-hot [128, slot, blk, value] (dense for the PE); the DVE
            # build reads qv's duplicated pairs with a [1,2] last dim so
            # is_equal qualifies for the 2x perf mode.  One instruction
            # per blk keeps the APs at 3 free dims.
            def build_window(w):
                nslots = min(W_SL, N_SLOT - w * W_SL)
                t = ohp.tile([128, W_SL, 2, 256], bf16, tag="ohw")
                t0 = t[:]
                for b in range(2):
                    out = rawap(t0, b * 256,
                                [t0.ap[0], [512, nslots], [2, 128], [1, 2]])
                    in0 = rawap(qv0, b * 1040 + w * W_SL * 2,
                                [qv0.ap[0], [2, nslots], [0, 128], [1, 2]])
                    in1 = rawap(negio[:], 0,
                                [negio[:].ap[0], [0, nslots], [2, 128],
                                 [1, 2]])
                    nc.vector.tensor_tensor(out=out, in0=in0, in1=in1,
                                            op=OP.is_equal)
                ohw[w] = t

            def chunk_mm(t):
                w1_, s1 = divmod(t, W_SL)
                w2_, s2 = divmod(t + 1, W_SL)
                stop = t == 511
                if t == 255:
                    rhs = ohw[w2_][:, s2, 1, :]
                    for h in range(2):
                        lhsT = ohw[w1_][:, s1, 0, h * 128:(h + 1) * 128]
                        nc.tensor.matmul(G[h][:, 256:512], lhsT, rhs,
                                         start=False, stop=False)
                    return
                rhs = ohw[w2_][:, s2, :, :]
                for h in range(2):
                    lhsT = ohw[w1_][:, s1, 0, h * 128:(h + 1) * 128]
                    nc.tensor.matmul(G[h][:], lhsT, rhs,
                                     start=first[h], stop=stop)
                    first[h] = False

            for w in range(n_win):
                build_window(w)
                lo = max(0, w * W_SL - 1)
                hi = min(512, w * W_SL + W_SL - 1)
                for t in range(lo, hi):
                    chunk_mm(t)

            # S = G_h + G_v ; co = S + S^T
            s_half = []
            for h in range(2):
                tmp = gl.tile([128, 256], f32, tag="stmp")
                nc.scalar.activation(out=tmp[:], in_=G[h][:, 0:256],
                                     func=AF.Copy)
                s = gl.tile([128, 256], f32, tag=f"s{h}")
                nc.vector.tensor_tensor(out=s[:], in0=G[h][:, 256:512],
                                        in1=tmp[:], op=OP.add)
                s_half.append(s)
            co_t = gl.tile([128, 2, 256], bf16, tag="co")
            for h in range(2):
                tp = psc.tile([128, 256], f32, tag="pc")
                for j in range(2):
                    nc.tensor.matmul(tp[:, j * 128:(j + 1) * 128],
                                     s_half[j][:, h * 128:(h + 1) * 128],
                                     id128[:], start=True, stop=True)
                nc.vector.tensor_tensor(out=co_t[:, h, :], in0=tp[:],
                                        in1=s_half[h][:], op=OP.add)
            if debug:
                nc.sync.dma_start(
                    out=dbg["co"][img * n_ch + ch],
                    in_=co_t[:].rearrange("p a b -> p (a b)"))

            # 9 tap-shifted replicas into the conv1 im2col buffer
            P_IMC = IMC_R * IMC_C
            for dy in range(3):
                for dx in range(3):
                    tpart = (dy * 3 + dx) * 3 + ch
                    off = tpart * P_IMC + (2 - dy) * IMC_C + (2 - dx)
                    nc.gpsimd.dma_start(
                        out=rawap(imcb[:], off,
                                  [[IMC_C, 128], [128 * IMC_C, 2], [1, 256]]),
                        in_=co_t[:])

        if n_ch < 3:
            continue   # debug mode without convs

        # ---------- conv1 (27 -> 32, relu) ----------
        for band in range(16):
            y0 = band * 16
            bt1 = cv.tile([27, 16, 260], bf16, tag="cvA")
            nc.sync.dma_start(out=bt1[:], in_=imcb[:, y0 + 1:y0 + 17, :])
            st1 = cv.tile([32, 16, 256], bf16, tag="cvB")
            for ci in range(8):
                p1 = psc.tile([32, 512], f32, tag="pc")
                nc.tensor.matmul(p1[:], w1sb[:],
                                 bt1[:, 2 * ci:2 * ci + 2, 1:257],
                                 start=True, stop=True)
                nc.scalar.activation(out=st1[:, 2 * ci:2 * ci + 2, :],
                                     in_=p1[:], func=AF.Relu, bias=b1sb[:])
            nc.gpsimd.dma_start(out=c1ob[:, 2 + y0:2 + y0 + 16, 2:258],
                                in_=st1[:])
        # ---------- conv2 (32 -> 32, 5x5, pool) ----------
        pf2 = big.tile([32, 128, 128], bf16, tag="pf")
        for band in range(32):
            y0 = band * 8
            bt2 = cv.tile([128, 12, 260], bf16, tag="cvA")
            for s in range(4):
                nc.sync.dma_start(out=bt2[s * 32:(s + 1) * 32, :, :],
                                  in_=rawap(
                    c1ob[:], y0 * C1O_C + s,
                    [[C1O_R * C1O_C, 32], [C1O_C, 12], [1, 260]]))
            rft2 = cv.tile([32, 8, 256], bf16, tag="cvB")
            for ci in range(2):
                p2 = psc.tile([128, 256], f32, tag="pc")
                for rr in range(8):
                    for dxg in range(2):
                        nc.tensor.matmul(
                            p2[:], w2sb[:, rr, dxg, :],
                            bt2[:, 4 * ci + rr, 4 * dxg:4 * dxg + 256],
                            start=(rr == 0 and dxg == 0),
                            stop=(rr == 7 and dxg == 1))
                bsb2 = cv.tile([128, 256], bf16, tag="cvC")
                nc.scalar.activation(out=bsb2[:], in_=p2[:],
                                     func=AF.Identity, bias=b2sb[:])
                nc.gpsimd.dma_start(
                    out=rawap(rft2[:], ci * 1024,
                              [rft2[:].ap[0], [256, 4], [1, 256]]),
                    in_=bsb2[:])
            tmp2 = cv.tile([32, 4, 256], bf16, tag="cvD")
            nc.vector.tensor_tensor(
                out=tmp2[:],
                in0=rawap(rft2[:], 0, [rft2[:].ap[0], [512, 4], [1, 256]]),
                in1=rawap(rft2[:], 256, [rft2[:].ap[0], [512, 4], [1, 256]]),
                op=OP.max)
            nc.vector.tensor_tensor(
                out=pf2[:, band * 4:band * 4 + 4, :],
                in0=rawap(tmp2[:], 0, [tmp2[:].ap[0], [256, 4], [2, 128]]),
                in1=rawap(tmp2[:], 1, [tmp2[:].ap[0], [256, 4], [2, 128]]),
                op=OP.max)
        if debug and img == n_img - 1:
            nc.sync.dma_start(out=dbg["pf2"][:], in_=pf2[:])

        # ---------- conv3 input (4-shift replicate, pad 1) ----------
        c3in = big.tile([128, 130, 132], bf16, tag="cio")
        nc.vector.memset(c3in[:], 0.0)
        for s in range(4):
            c0 = max(0, 1 - s)
            cN = min(132, 129 - s)
            nc.gpsimd.dma_start(
                out=c3in[s * 32:(s + 1) * 32, 1:129, c0:cN],
                in_=pf2[:, :, c0 - 1 + s:cN - 1 + s])

        # ---------- conv3 (32 -> 64, 3x3, relu) ----------
        pf3 = big.tile([64, 128, 128], bf16, tag="pf")
        for ci in range(32):
            y = ci * 4
            p3 = psc.tile([64, 512], f32, tag="pc")
            for dy in range(3):
                nc.tensor.matmul(p3[:], w3sb[:, dy, :],
                                 c3in[:, y + dy:y + dy + 4, 0:128],
                                 start=(dy == 0), stop=(dy == 2))
            nc.scalar.activation(out=pf3[:, y:y + 4, :], in_=p3[:],
                                 func=AF.Relu, bias=b3sb[:])
        if debug and img == n_img - 1:
            nc.sync.dma_start(out=dbg["pf3"][:], in_=pf3[:])

        # ---------- conv4 input (2-shift replicate, pad 2) ----------
        c4in = big.tile([128, 132, 132], bf16, tag="cio")
        nc.vector.memset(c4in[:], 0.0)
        for s in range(2):
            c0 = max(0, 2 - s)
            cN = min(132, 130 - s)
            nc.gpsimd.dma_start(
                out=c4in[s * 64:(s + 1) * 64, 2:130, c0:cN],
                in_=pf3[:, :, c0 - 2 + s:cN - 2 + s])

        # ---------- conv4 (64 -> 64, 5x5, pool) ----------
        pf4 = big.tile([64, 64, 64], bf16, tag="pf")
        for grp in range(16):     # 16 groups of 4 chunks (2 rows each)
            rft4 = cv.tile([64, 8, 128], bf16, tag="cvB")
            for cj in range(4):
                ci = grp * 4 + cj
                y = ci * 2
                p4 = psc.tile([128, 128], f32, tag="pc")
                for rr in range(6):
                    for dxg in range(3):
                        nc.tensor.matmul(
                            p4[:], w4sb[:, rr, dxg, :],
                            c4in[:, y + rr, 2 * dxg:2 * dxg + 128],
                            start=(rr == 0 and dxg == 0),
                            stop=(rr == 5 and dxg == 2))
                bsb4 = cv.tile([128, 128], bf16, tag="cvC")
                nc.scalar.activation(out=bsb4[:], in_=p4[:],
                                     func=AF.Identity, bias=b4sb[:])
                nc.gpsimd.dma_start(
                    out=rawap(rft4[:], cj * 256,
                              [rft4[:].ap[0], [128, 2], [1, 128]]),
                    in_=bsb4[:])
            tmp4 = cv.tile([64, 4, 128], bf16, tag="cvD")
            nc.vector.tensor_tensor(
                out=tmp4[:],
                in0=rawap(rft4[:], 0, [rft4[:].ap[0], [256, 4], [1, 128]]),
                in1=rawap(rft4[:], 128, [rft4[:].ap[0], [256, 4], [1, 128]]),
                op=OP.max)
            nc.vector.tensor_tensor(
                out=pf4[:, grp * 4:grp * 4 + 4, :],
                in0=rawap(tmp4[:], 0, [tmp4[:].ap[0], [128, 4], [2, 64]]),
                in1=rawap(tmp4[:], 1, [tmp4[:].ap[0], [128, 4], [2, 64]]),
                op=OP.max)
        if debug and img == n_img - 1:
            nc.sync.dma_start(out=dbg["pf4"][:], in_=pf4[:])

        # ---------- conv5 input (2-shift replicate, pad 1) ----------
        c5in = big.tile([128, 66, 68], bf16, tag="cio")
        nc.vector.memset(c5in[:], 0.0)
        for s in range(2):
            c0 = max(0, 1 - s)
            cN = min(68, 65 - s)
            nc.gpsimd.dma_start(
                out=c5in[s * 64:(s + 1) * 64, 1:65, c0:cN],
                in_=pf4[:, :, c0 - 1 + s:cN - 1 + s])

        # ---------- conv5 (64 -> 128, 3x3, relu) ----------
        c6in = big.tile([128, 68, 72], bf16, tag="c6in")
        nc.vector.memset(c6in[:], 0.0)
        for ci in range(16):
            y = ci * 4
            p5 = psc.tile([128, 256], f32, tag="pc")
            for dy in range(3):
                for dxg in range(2):
                    nc.tensor.matmul(
                        p5[:], w5sb[:, dy, dxg, :],
                        rawap(c5in[:], (y + dy) * 68 + 2 * dxg,
                              [c5in[:].ap[0], [68, 4], [1, 64]]),
                        start=(dy == 0 and dxg == 0),
                        stop=(dy == 2 and dxg == 1))
            nc.scalar.activation(out=c6in[:, 2 + y:2 + y + 4, 2:66],
                                 in_=p5[:], func=AF.Relu, bias=b5sb[:])
        if debug and img == n_img - 1:
            nc.sync.dma_start(out=dbg["c6in"][:], in_=c6in[:])

        # ---------- conv6 (128 -> 128, 5x5, pool) ----------
        for ci in range(8):
            y = ci * 8
            p6 = psc.tile([128, 512], f32, tag="pc")
            for dy in range(5):
                for dx in range(5):
                    nc.tensor.matmul(
                        p6[:], w6sb[:, dy, dx, :],
                        rawap(c6in[:], (y + dy) * 72 + dx,
                              [c6in[:].ap[0], [72, 8], [1, 64]]),
                        start=(dy == 0 and dx == 0),
                        stop=(dy == 4 and dx == 4))
            sb6 = cv.tile([128, 8, 64], bf16, tag="cvC")
            nc.scalar.activation(out=sb6[:], in_=p6[:], func=AF.Identity,
                                 bias=b6sb[:])
            t6 = cv.tile([128, 4, 64], bf16, tag="cvD")
            nc.vector.tensor_tensor(
                out=t6[:],
                in0=rawap(sb6[:], 0, [sb6[:].ap[0], [128, 4], [1, 64]]),
                in1=rawap(sb6[:], 64, [sb6[:].ap[0], [128, 4], [1, 64]]),
                op=OP.max)
            hout = rawap(h_sb[:], img * 1024 + ci * 128,
                         [h_sb[:].ap[0], [32, 4], [1, 32]])
            nc.vector.tensor_tensor(
                out=hout,
                in0=rawap(t6[:], 0, [t6[:].ap[0], [64, 4], [2, 32]]),
                in1=rawap(t6[:], 1, [t6[:].ap[0], [64, 4], [2, 32]]),
                op=OP.max)

    if n_ch == 3:
        if debug:
            nc.sync.dma_start(out=dbg["h"][:], in_=h_sb[:])
        # ================= fc layers =================
        FW1_BLK = 8
        for kb in range(1024 // FW1_BLK):
            fwt = cv.tile([128, FW1_BLK, 256], bf16, tag="fwt")
            nc.sync.dma_start(
                out=fwt[:],
                in_=rawap(env["fw1p"], kb * FW1_BLK * 128 * 256,
                          [[256, 128], [128 * 256, FW1_BLK], [1, 256]]))
            for j in range(FW1_BLK):
                s = kb * FW1_BLK + j
                nc.tensor.matmul(pfc1[:], h_sb[:, :, s], fwt[:, j, :],
                                 start=(s == 0), stop=(s == 1023))

        h1 = singles.tile([NI, 256], f32, tag="h1")
        nc.vector.tensor_tensor(out=h1[:], in0=pfc1[:], in1=fb1sb[:NI, :],
                                op=OP.add)
        nc.vector.tensor_scalar_max(h1[:], h1[:], 0.0)
        if debug:
            nc.sync.dma_start(out=dbg["h1"][:NI, :], in_=h1[:])

        h1T = singles.tile([128, 2, NI], bf16, tag="h1T")
        for j in range(2):
            ptp = psc.tile([128, NI], f32, tag="pc")
            nc.tensor.matmul(ptp[:], h1[:, j * 128:(j + 1) * 128],
                             id4[:NI, :NI], start=True, stop=True)
            nc.scalar.activation(out=h1T[:, j, :], in_=ptp[:], func=AF.Copy)

        pfc2 = psc.tile([NI, 256], f32, tag="pc")
        for j in range(2):
            nc.tensor.matmul(pfc2[:], h1T[:, j, :], fw2sb[:, j, :],
                             start=(j == 0), stop=(j == 1))
        h2 = singles.tile([NI, 256], f32, tag="h2")
        nc.vector.tensor_tensor(out=h2[:], in0=pfc2[:], in1=fb2sb[:NI, :],
                                op=OP.add)
        nc.vector.tensor_scalar_max(h2[:], h2[:], 0.0)

        h2T = singles.tile([128, 2, NI], bf16, tag="h2T")
        for j in range(2):
            ptp = psc.tile([128, NI], f32, tag="pc")
            nc.tensor.matmul(ptp[:], h2[:, j * 128:(j + 1) * 128],
                             id4[:NI, :NI], start=True, stop=True)
            nc.scalar.activation(out=h2T[:, j, :], in_=ptp[:], func=AF.Copy)

        pfc3 = psc.tile([NI, 1], f32, tag="pc")
        for j in range(2):
            nc.tensor.matmul(pfc3[:], h2T[:, j, :], fw3sb[:, j, :],
                             start=(j == 0), stop=(j == 1))
        osb = singles.tile([NI, 1], f32, tag="osb")
        nc.scalar.activation(out=osb[:], in_=pfc3[:], func=AF.Sigmoid,
                             bias=fb3sb[:NI, :])
        nc.sync.dma_start(out=out4, in_=osb[:])
    else:
        # tiny debug build: just write something to out4
        osb = singles.tile([4, 1], f32, tag="osb")
        nc.vector.memset(osb[:], 0.0)
        nc.sync.dma_start(out=out4, in_=osb[:n_img, :])

    ctx.close()


def kernel(**inputs):
    from concourse.bass_utils import run_bass_kernel_spmd

    inputs = dict(inputs)
    debug = bool(inputs.pop("_debug", False))
    trace = bool(inputs.pop("_trace", False))
    key = ("k", debug)
    if key not in _BUILD_CACHE:
        _BUILD_CACHE[key] = _build(debug=debug)
    nc = _BUILD_CACHE[key]

    packed = _pack_weights(inputs)
    x = np.asarray(inputs["x"], np.float32)
    in_maps = []
    for c in range(N_CORES):
        m = dict(packed)
        m["x4"] = np.ascontiguousarray(x[c * N_IMG:(c + 1) * N_IMG])
        in_maps.append(m)

    res = run_bass_kernel_spmd(nc, in_maps, core_ids=list(range(N_CORES)),
                               trace=trace)
    out = np.concatenate([res.results[c]["out4"] for c in range(N_CORES)],
                         axis=0)
    kernel._last_results = res
    return out



# revision 27
# speedup vs baseline: 2.5238x; 1.0361x over previous
"""Trainium2 Bass kernel for nn_CoOccurrenceMatrixFast.

GLCM (256x256-bin co-occurrence histograms) via one-hot matmuls on the PE
(exact integer counts accumulated in fp32 PSUM), followed by the 6-conv +
3-fc CNN in bf16 with fp32 accumulation.  Pure data parallel: batch 32
sharded as 4 images per NeuronCore across 8 cores.

kernel(**inputs) takes the full unsharded inputs, returns the full [32, 1].
"""

import numpy as np
import ml_dtypes

N_CORES = 8
N_IMG = 4          # images per core
N_CH = 3
NB = 256
MEAN = [0.485, 0.456, 0.406]
STD = [0.229, 0.224, 0.225]
PAD_SENTINEL = 384.0   # never equals -q (q in 0..255); exact in bf16

IMC_R, IMC_C = 258, 260        # conv1 im2col dram buffer [27, 258, 260]
C1O_R, C1O_C = 260, 264        # conv1 output dram buffer [32, 260, 264]

W_SL = 6                       # one-hot window slots
N_SLOT = 513

_BUILD_CACHE = {}


def _pack_weights(inp):
    f32 = np.float32
    bf16 = ml_dtypes.bfloat16
    w1, w2, w3, w4, w5, w6 = (np.asarray(inp[k], f32) for k in
                              ("w1", "w2", "w3", "w4", "w5", "w6"))
    out = {}

    # conv1: lhsT [27, 32];  K row t=(dy*3+dx)*3+ch
    w1p = np.zeros((27, 32), f32)
    for dy in range(3):
        for dx in range(3):
            for ch in range(3):
                w1p[(dy * 3 + dx) * 3 + ch, :] = w1[:, ch, dy, dx]
    out["w1p"] = w1p.astype(bf16)

    # conv2: [8, 2, 128K, 128M]; K=(s*32+ic); M=(oc*4+g)
    w2p = np.zeros((8, 2, 128, 128), f32)
    for r in range(8):
        for g in range(4):
            dy = r - g
            if not (0 <= dy < 5):
                continue
            for s in range(4):
                w2p[r, 0, s * 32:(s + 1) * 32, g::4] = w2[:, :, dy, s].T
            w2p[r, 1, 0:32, g::4] = w2[:, :, dy, 4].T
    out["w2p"] = w2p.astype(bf16)

    # conv3: [3, 128K, 64M]; K=(s*32+ic), s in 0..2
    w3p = np.zeros((3, 128, 64), f32)
    for dy in range(3):
        for s in range(3):
            w3p[dy, s * 32:(s + 1) * 32, :] = w3[:, :, dy, s].T
    out["w3p"] = w3p.astype(bf16)

    # conv4: [6, 3, 128K, 128M]; K=(s*64+ic); M=(oc*2+g)
    w4p = np.zeros((6, 3, 128, 128), f32)
    for r in range(6):
        for g in range(2):
            dy = r - g
            if not (0 <= dy < 5):
                continue
            for s in range(2):
                w4p[r, 0, s * 64:(s + 1) * 64, g::2] = w4[:, :, dy, s].T
                w4p[r, 1, s * 64:(s + 1) * 64, g::2] = w4[:, :, dy, 2 + s].T
            w4p[r, 2, 0:64, g::2] = w4[:, :, dy, 4].T
    out["w4p"] = w4p.astype(bf16)

    # conv5: [3, 2, 128K, 128M]; K=(s*64+ic)
    w5p = np.zeros((3, 2, 128, 128), f32)
    for dy in range(3):
        for s in range(2):
            w5p[dy, 0, s * 64:(s + 1) * 64, :] = w5[:, :, dy, s].T
        w5p[dy, 1, 0:64, :] = w5[:, :, dy, 2].T
    out["w5p"] = w5p.astype(bf16)

    # conv6: [5, 5, 128K(ic), 128M(oc)]
    out["w6p"] = np.ascontiguousarray(w6.transpose(2, 3, 1, 0)).astype(bf16)

    out["b1p"] = np.asarray(inp["b1"], f32).reshape(32, 1)
    out["b2p"] = np.repeat(np.asarray(inp["b2"], f32), 4).reshape(128, 1)
    out["b3p"] = np.asarray(inp["b3"], f32).reshape(64, 1)
    out["b4p"] = np.repeat(np.asarray(inp["b4"], f32), 2).reshape(128, 1)
    out["b5p"] = np.asarray(inp["b5"], f32).reshape(128, 1)
    out["b6p"] = np.asarray(inp["b6"], f32).reshape(128, 1)

    fw1 = np.asarray(inp["fw1"], f32).reshape(256, 128, 1024)     # [o, ch, s]
    out["fw1p"] = np.ascontiguousarray(fw1.transpose(2, 1, 0)).astype(bf16)
    fw2 = np.asarray(inp["fw2"], f32)
    out["fw2p"] = np.ascontiguousarray(fw2.T.reshape(2, 128, 256)).astype(bf16)
    fw3 = np.asarray(inp["fw3"], f32)
    out["fw3p"] = np.ascontiguousarray(fw3.T.reshape(2, 128, 1)).astype(bf16)

    out["fb1bc"] = np.tile(np.asarray(inp["fb1"], f32).reshape(1, 256), (4, 1))
    out["fb2bc"] = np.tile(np.asarray(inp["fb2"], f32).reshape(1, 256), (4, 1))
    out["fb3bc"] = np.tile(np.asarray(inp["fb3"], f32).reshape(1, 1), (4, 1))

    qc = np.zeros((128, 6), f32)
    for c in range(3):
        qc[:, c] = MEAN[c]
        qc[:, 3 + c] = STD[c]
    out["qconst"] = qc

    out["negiota"] = np.tile((-np.arange(256, dtype=f32)).reshape(1, 256),
                             (128, 1)).astype(bf16)
    out["id128"] = np.eye(128, dtype=f32)
    out["id4"] = np.eye(4, dtype=f32)
    return out


def _build(debug=False, n_img=N_IMG, n_ch=N_CH):
    import concourse.bass as bass
    import concourse.tile as tile
    import concourse.mybir as mybir
    from concourse import bacc

    dt = mybir.dt
    f32, bf16 = dt.float32, dt.bfloat16

    nc = bacc.Bacc("TRN2", target_bir_lowering=False, debug=False,
                   num_devices=N_CORES)

    env = {}

    def din(name, shape, dtype):
        ap = nc.dram_tensor(name, shape, dtype, kind="ExternalInput").ap()
        env[name] = ap
        return ap

    din("x4", [n_img, N_CH, 256, 256], f32)
    din("w1p", [27, 32], bf16)
    din("w2p", [8, 2, 128, 128], bf16)
    din("w3p", [3, 128, 64], bf16)
    din("w4p", [6, 3, 128, 128], bf16)
    din("w5p", [3, 2, 128, 128], bf16)
    din("w6p", [5, 5, 128, 128], bf16)
    for nm, p in [("b1p", 32), ("b2p", 128), ("b3p", 64), ("b4p", 128),
                  ("b5p", 128), ("b6p", 128)]:
        din(nm, [p, 1], f32)
    din("fw1p", [1024, 128, 256], bf16)
    din("fw2p", [2, 128, 256], bf16)
    din("fw3p", [2, 128, 1], bf16)
    din("fb1bc", [4, 256], f32)
    din("fb2bc", [4, 256], f32)
    din("fb3bc", [4, 1], f32)
    din("negiota", [128, 256], bf16)
    din("qconst", [128, 6], f32)
    din("id128", [128, 128], f32)
    din("id4", [4, 4], f32)

    env["out4"] = nc.dram_tensor("out4", [n_img, 1], f32,
                                 kind="ExternalOutput").ap()
    dbg = {}
    if debug:
        def dout(name, shape, dtype=bf16):
            dbg[name] = nc.dram_tensor("dbg_" + name, shape, dtype,
                                       kind="ExternalOutput").ap()
        dout("co", [n_img * n_ch, 128, 512])
        dout("pf2", [32, 128, 128])
        dout("pf3", [64, 128, 128])
        dout("pf4", [64, 64, 64])
        dout("c6in", [128, 68, 72])
        dout("h", [128, n_img, 1024])
        dout("h1", [4, 256], f32)
    env["dbg"] = dbg

    with tile.TileContext(nc) as tc:
        _emit(nc, tc, bass, mybir, env, debug, n_img, n_ch)
    nc.compile()
    return nc


def _emit(nc, tc, bass, mybir, env, debug, n_img, n_ch):
    from contextlib import ExitStack
    dt = mybir.dt
    f32, bf16 = dt.float32, dt.bfloat16
    AF = mybir.ActivationFunctionType
    OP = mybir.AluOpType
    x4 = env["x4"]
    out4 = env["out4"]
    dbg = env["dbg"]

    def rawap(base, extra_off, dims):
        return bass.AP(tensor=base.tensor, offset=base.offset + extra_off,
                       ap=dims)

    ctx = ExitStack()
    singles = ctx.enter_context(tc.tile_pool(name="singles", bufs=1))
    dram = ctx.enter_context(tc.tile_pool(name="dram", bufs=1, space="DRAM"))
    psg = ctx.enter_context(tc.tile_pool(name="psg", bufs=1, space="PSUM"))
    psc = ctx.enter_context(tc.tile_pool(name="psc", bufs=2, space="PSUM"))
    psct = ctx.enter_context(tc.tile_pool(name="psct", bufs=2, space="PSUM"))
    gl = ctx.enter_context(tc.tile_pool(name="gl", bufs=2))
    ohp = ctx.enter_context(tc.tile_pool(name="ohp", bufs=2))
    cv = ctx.enter_context(tc.tile_pool(name="cv", bufs=2))
    big = ctx.enter_context(tc.tile_pool(name="big", bufs=1))

    # ---------------- constants to SBUF ----------------
    def load_const(name, shape, dtype, tag=None):
        t = singles.tile(shape, dtype, tag=tag or name)
        nc.sync.dma_start(out=t[:], in_=env[name])
        return t

    def load_w(dname, ntile_shape, dtype):
        # dram [T..., 128K, M] -> sbuf [128K, T..., M]
        sb = singles.tile(ntile_shape, dtype, tag=dname + "sb")
        d = env[dname]
        K = d.ap[-2][1]
        M = d.ap[-1][1]
        nt = 1
        for s, c in d.ap[:-2]:
            nt *= c
        in_dims = [[d.ap[-2][0], K], [K * M, nt], [1, M]]
        nc.sync.dma_start(out=sb[:], in_=rawap(d, 0, in_dims))
        return sb

    w1sb = load_const("w1p", [27, 32], bf16)
    w2sb = load_w("w2p", [128, 8, 2, 128], bf16)
    w3sb = load_w("w3p", [128, 3, 64], bf16)
    w4sb = load_w("w4p", [128, 6, 3, 128], bf16)
    w5sb = load_w("w5p", [128, 3, 2, 128], bf16)
    w6sb = load_w("w6p", [128, 5, 5, 128], bf16)
    fw2sb = load_w("fw2p", [128, 2, 256], bf16)
    fw3sb = load_w("fw3p", [128, 2, 1], bf16)

    b1sb = load_const("b1p", [32, 1], f32)
    b2sb = load_const("b2p", [128, 1], f32)
    b3sb = load_const("b3p", [64, 1], f32)
    b4sb = load_const("b4p", [128, 1], f32)
    b5sb = load_const("b5p", [128, 1], f32)
    b6sb = load_const("b6p", [128, 1], f32)
    fb1sb = load_const("fb1bc", [4, 256], f32)
    fb2sb = load_const("fb2bc", [4, 256], f32)
    fb3sb = load_const("fb3bc", [4, 1], f32)
    negio = load_const("negiota", [128, 256], bf16)
    qcsb = load_const("qconst", [128, 6], f32)
    id128 = load_const("id128", [128, 128], f32)
    id4 = load_const("id4", [4, 4], f32)

    # ---------------- DRAM scratch (zero-init once) ----------------
    imc = [dram.tile([27, IMC_R, IMC_C], bf16, tag=f"imc{i}", name=f"imc{i}")
           for i in range(2)]
    c1o = [dram.tile([32, C1O_R, C1O_C], bf16, tag=f"c1o{i}", name=f"c1o{i}")
           for i in range(2)]

    ZW = 512
    zt = singles.tile([128, ZW], bf16, tag="zeros")
    nc.vector.memset(zt[:], 0.0)
    ZN = 128 * ZW
    for buf, total in [(imc[0][:], 27 * IMC_R * IMC_C),
                       (imc[1][:], 27 * IMC_R * IMC_C),
                       (c1o[0][:], 32 * C1O_R * C1O_C),
                       (c1o[1][:], 32 * C1O_R * C1O_C)]:
        off = 0
        while off < total:
            n = min(ZN, total - off)
            p = n // ZW
            if p >= 1:
                nc.gpsimd.dma_start(
                    out=rawap(buf, off, [[ZW, p], [1, ZW]]),
                    in_=zt[:p, :ZW])
                off += p * ZW
            else:
                nc.gpsimd.dma_start(out=rawap(buf, off, [[n, 1], [1, n]]),
                                    in_=zt[:1, :n])
                off += n

    h_sb = singles.tile([128, n_img, 1024], bf16, tag="h")
    NI = n_img
    pfc1 = psg.tile([NI, 256], f32, tag="pfc1")

    # ================= pipelined streams =================
    # The PE drains its queue strictly in order, so conv(i) stalls idle
    # it even though GLCM(i+1) matmuls are data-ready.  Emitting conv(i)
    # and GLCM(i+1) interleaved keeps the PE stream dense (and the HAM
    # clock-gate warm).
    def glcm_stream(img):
        imcb = imc[img % 2]

        for ch in range(n_ch):
            xbase = x4[img, ch]    # [256, 256] dram ap

            xin = gl.tile([128, 2, 256], f32, tag="xin")
            nc.sync.dma_start(out=xin[:], in_=rawap(
                xbase, 0, [[256, 128], [128 * 256, 2], [1, 256]]))
            xin2 = gl.tile([128, 2, 256], f32, tag="xin2")
            nc.sync.dma_start(out=xin2[:, 0, :], in_=rawap(
                xbase, 256, [[256, 128], [1, 256]]))
            nc.vector.memset(xin2[:, 1, :], 0.0)
            nc.sync.dma_start(out=xin2[:127, 1, :], in_=rawap(
                xbase, 129 * 256, [[256, 127], [1, 256]]))

            # quantize: u = min(relu(std*x+mean), 1)*255; store -floor(u).
            # Each value is stored TWICE ([s,2] pairs) so the one-hot
            # is_equal can read it with a last-dim [1,2] stride-1 AP,
            # qualifying for the DVE 2x perf mode.
            qv = gl.tile([128, 2, 520, 2], bf16, tag="qv")
            qv0 = qv[:]
            nc.vector.memset(qv[:, 1, :, :], PAD_SENTINEL)
            nc.vector.memset(qv[:, 0, 512:513, :], PAD_SENTINEL)
            for (src, dst_off, blk_parts) in (
                    (xin, 0, (128, 128)), (xin2, 520 + 1, (128, 127))):
                r = gl.tile([128, 2, 256], f32, tag="qr")
                nc.scalar.activation(out=r[:], in_=src[:], func=AF.Relu,
                                     bias=qcsb[:, ch:ch + 1],
                                     scale=qcsb[:, 3 + ch:3 + ch + 1])
                u = gl.tile([128, 2, 256], f32, tag="qu")
                nc.vector.tensor_scalar(out=u[:], in0=r[:], scalar1=1.0,
                                        scalar2=float(NB - 1), op0=OP.min,
                                        op1=OP.mult)
                # HW f32->int cast rounds to nearest; shift so that
                # round(u - 0.5 + eps) == floor(u) incl. integer ties.
                u2 = gl.tile([128, 2, 256], f32, tag="qu2")
                nc.vector.tensor_scalar(out=u2[:], in0=u[:], scalar1=0.5,
                                        scalar2=float(2.0 ** -15),
                                        op0=OP.subtract, op1=OP.add)
                qi = gl.tile([128, 2, 256], dt.int32, tag="qi")
                nc.vector.tensor_copy(out=qi[:], in_=u2[:])
                for blk in range(2):
                    np_ = blk_parts[blk]
                    L = dst_off + 256 * blk
                    noff = (L // 520) * 1040 + (L % 520) * 2
                    dst = rawap(qv0, noff,
                                [[qv0.ap[0][0], np_], [2, 256], [1, 2]])
                    qin = rawap(qi[:], 256 * blk,
                                [[qi[:].ap[0][0], np_], [1, 256], [0, 2]])
                    nc.vector.tensor_scalar(out=dst, in0=qin, scalar1=-1.0,
                                            scalar2=None, op0=OP.mult)
            yield

            # G psum: [128, 512] = [G_h | G_v] per left-half
            G = [psg.tile([128, 512], f32, tag=f"G{h}", name=f"G{h}")
                 for h in range(2)]

            n_win = (N_SLOT + W_SL - 1) // W_SL
            ohw = [None] * n_win
            first = [True, True]

            # one-hot [128, slot, blk, value] (dense for the PE); the DVE
            # build reads qv's duplicated pairs with a [1,2] last dim so
            # is_equal qualifies for the 2x perf mode.  One instruction
            # per blk keeps the APs at 3 free dims.
            def build_window(w):
                nslots = min(W_SL, N_SLOT - w * W_SL)
                t = ohp.tile([128, W_SL, 2, 256], bf16, tag="ohw")
                t0 = t[:]
                for b in range(2):
                    out = rawap(t0, b * 256,
                                [t0.ap[0], [512, nslots], [2, 128], [1, 2]])
                    in0 = rawap(qv0, b * 1040 + w * W_SL * 2,
                                [qv0.ap[0], [2, nslots], [0, 128], [1, 2]])
                    in1 = rawap(negio[:], 0,
                                [negio[:].ap[0], [0, nslots], [2, 128],
                                 [1, 2]])
                    nc.vector.tensor_tensor(out=out, in0=in0, in1=in1,
                                            op=OP.is_equal)
                ohw[w] = t

            def chunk_mm(t):
                w1_, s1 = divmod(t, W_SL)
                w2_, s2 = divmod(t + 1, W_SL)
                stop = t == 511
                if t == 255:
                    rhs = ohw[w2_][:, s2, 1, :]
                    for h in range(2):
                        lhsT = ohw[w1_][:, s1, 0, h * 128:(h + 1) * 128]
                        nc.tensor.matmul(G[h][:, 256:512], lhsT, rhs,
                                         start=False, stop=False)
                    return
                rhs = ohw[w2_][:, s2, :, :]
                for h in range(2):
                    lhsT = ohw[w1_][:, s1, 0, h * 128:(h + 1) * 128]
                    nc.tensor.matmul(G[h][:], lhsT, rhs,
                                     start=first[h], stop=stop)
                    first[h] = False

            for w in range(n_win):
                build_window(w)
                lo = max(0, w * W_SL - 1)
                hi = min(512, w * W_SL + W_SL - 1)
                for t in range(lo, hi):
                    chunk_mm(t)
                yield

            # S = G_h + G_v ; co = S + S^T
            s_half = []
            for h in range(2):
                tmp = gl.tile([128, 256], f32, tag="stmp")
                nc.scalar.activation(out=tmp[:], in_=G[h][:, 0:256],
                                     func=AF.Copy)
                s = gl.tile([128, 256], f32, tag=f"s{h}")
                nc.vector.tensor_tensor(out=s[:], in0=G[h][:, 256:512],
                                        in1=tmp[:], op=OP.add)
                s_half.append(s)
            co_t = gl.tile([128, 2, 256], bf16, tag="co")
            for h in range(2):
                tp = psct.tile([128, 256], f32, tag="pt")
                for j in range(2):
                    nc.tensor.matmul(tp[:, j * 128:(j + 1) * 128],
                                     s_half[j][:, h * 128:(h + 1) * 128],
                                     id128[:], start=True, stop=True)
                nc.vector.tensor_tensor(out=co_t[:, h, :], in0=tp[:],
                                        in1=s_half[h][:], op=OP.add)
            if debug:
                nc.sync.dma_start(
                    out=dbg["co"][img * n_ch + ch],
                    in_=co_t[:].rearrange("p a b -> p (a b)"))

            # 9 tap-shifted replicas into the conv1 im2col buffer
            P_IMC = IMC_R * IMC_C
            for dy in range(3):
                for dx in range(3):
                    tpart = (dy * 3 + dx) * 3 + ch
                    off = tpart * P_IMC + (2 - dy) * IMC_C + (2 - dx)
                    nc.gpsimd.dma_start(
                        out=rawap(imcb[:], off,
                                  [[IMC_C, 128], [128 * IMC_C, 2], [1, 256]]),
                        in_=co_t[:])
            yield

    def conv_stream(img):
        imcb = imc[img % 2]
        c1ob = c1o[img % 2]

        # ---------- conv1 (27 -> 32, relu) ----------
        for band in range(16):
            y0 = band * 16
            bt1 = cv.tile([27, 16, 260], bf16, tag="cvA")
            nc.sync.dma_start(out=bt1[:], in_=imcb[:, y0 + 1:y0 + 17, :])
            st1 = cv.tile([32, 16, 256], bf16, tag="cvB")
            for ci in range(8):
                p1 = psc.tile([32, 512], f32, tag="pc")
                nc.tensor.matmul(p1[:], w1sb[:],
                                 bt1[:, 2 * ci:2 * ci + 2, 1:257],
                                 start=True, stop=True)
                nc.scalar.activation(out=st1[:, 2 * ci:2 * ci + 2, :],
                                     in_=p1[:], func=AF.Relu, bias=b1sb[:])
            nc.gpsimd.dma_start(out=c1ob[:, 2 + y0:2 + y0 + 16, 2:258],
                                in_=st1[:])
            yield
        # ---------- conv2 (32 -> 32, 5x5, pool) ----------
        pf2 = big.tile([32, 128, 128], bf16, tag="pf")
        for band in range(32):
            y0 = band * 8
            bt2 = cv.tile([128, 12, 260], bf16, tag="cvA")
            for s in range(4):
                nc.sync.dma_start(out=bt2[s * 32:(s + 1) * 32, :, :],
                                  in_=rawap(
                    c1ob[:], y0 * C1O_C + s,
                    [[C1O_R * C1O_C, 32], [C1O_C, 12], [1, 260]]))
            rft2 = cv.tile([32, 8, 256], bf16, tag="cvB")
            for ci in range(2):
                p2 = psc.tile([128, 256], f32, tag="pc")
                for rr in range(8):
                    for dxg in range(2):
                        nc.tensor.matmul(
                            p2[:], w2sb[:, rr, dxg, :],
                            bt2[:, 4 * ci + rr, 4 * dxg:4 * dxg + 256],
                            start=(rr == 0 and dxg == 0),
                            stop=(rr == 7 and dxg == 1))
                bsb2 = cv.tile([128, 256], bf16, tag="cvC")
                nc.scalar.activation(out=bsb2[:], in_=p2[:],
                                     func=AF.Identity, bias=b2sb[:])
                nc.gpsimd.dma_start(
                    out=rawap(rft2[:], ci * 1024,
                              [rft2[:].ap[0], [256, 4], [1, 256]]),
                    in_=bsb2[:])
            tmp2 = cv.tile([32, 4, 256], bf16, tag="cvD")
            nc.vector.tensor_tensor(
                out=tmp2[:],
                in0=rawap(rft2[:], 0, [rft2[:].ap[0], [512, 4], [1, 256]]),
                in1=rawap(rft2[:], 256, [rft2[:].ap[0], [512, 4], [1, 256]]),
                op=OP.max)
            nc.vector.tensor_tensor(
                out=pf2[:, band * 4:band * 4 + 4, :],
                in0=rawap(tmp2[:], 0, [tmp2[:].ap[0], [256, 4], [2, 128]]),
                in1=rawap(tmp2[:], 1, [tmp2[:].ap[0], [256, 4], [2, 128]]),
                op=OP.max)
            yield
        if debug and img == n_img - 1:
            nc.sync.dma_start(out=dbg["pf2"][:], in_=pf2[:])

        # ---------- conv3 input (4-shift replicate, pad 1) ----------
        c3in = big.tile([128, 130, 132], bf16, tag="cio")
        nc.vector.memset(c3in[:], 0.0)
        for s in range(4):
            c0 = max(0, 1 - s)
            cN = min(132, 129 - s)
            nc.gpsimd.dma_start(
                out=c3in[s * 32:(s + 1) * 32, 1:129, c0:cN],
                in_=pf2[:, :, c0 - 1 + s:cN - 1 + s])
        yield

        # ---------- conv3 (32 -> 64, 3x3, relu) ----------
        pf3 = big.tile([64, 128, 128], bf16, tag="pf")
        for ci in range(32):
            y = ci * 4
            p3 = psc.tile([64, 512], f32, tag="pc")
            for dy in range(3):
                nc.tensor.matmul(p3[:], w3sb[:, dy, :],
                                 c3in[:, y + dy:y + dy + 4, 0:128],
                                 start=(dy == 0), stop=(dy == 2))
            nc.scalar.activation(out=pf3[:, y:y + 4, :], in_=p3[:],
                                 func=AF.Relu, bias=b3sb[:])
            if ci % 4 == 3:
                yield
        if debug and img == n_img - 1:
            nc.sync.dma_start(out=dbg["pf3"][:], in_=pf3[:])

        # ---------- conv4 input (2-shift replicate, pad 2) ----------
        c4in = big.tile([128, 132, 132], bf16, tag="cio")
        nc.vector.memset(c4in[:], 0.0)
        for s in range(2):
            c0 = max(0, 2 - s)
            cN = min(132, 130 - s)
            nc.gpsimd.dma_start(
                out=c4in[s * 64:(s + 1) * 64, 2:130, c0:cN],
                in_=pf3[:, :, c0 - 2 + s:cN - 2 + s])
        yield

        # ---------- conv4 (64 -> 64, 5x5, pool) ----------
        pf4 = big.tile([64, 64, 64], bf16, tag="pf")
        for grp in range(16):     # 16 groups of 4 chunks (2 rows each)
            rft4 = cv.tile([64, 8, 128], bf16, tag="cvB")
            for cj in range(4):
                ci = grp * 4 + cj
                y = ci * 2
                p4 = psc.tile([128, 128], f32, tag="pc")
                for rr in range(6):
                    for dxg in range(3):
                        nc.tensor.matmul(
                            p4[:], w4sb[:, rr, dxg, :],
                            c4in[:, y + rr, 2 * dxg:2 * dxg + 128],
                            start=(rr == 0 and dxg == 0),
                            stop=(rr == 5 and dxg == 2))
                bsb4 = cv.tile([128, 128], bf16, tag="cvC")
                nc.scalar.activation(out=bsb4[:], in_=p4[:],
                                     func=AF.Identity, bias=b4sb[:])
                nc.gpsimd.dma_start(
                    out=rawap(rft4[:], cj * 256,
                              [rft4[:].ap[0], [128, 2], [1, 128]]),
                    in_=bsb4[:])
            tmp4 = cv.tile([64, 4, 128], bf16, tag="cvD")
            nc.vector.tensor_tensor(
                out=tmp4[:],
                in0=rawap(rft4[:], 0, [rft4[:].ap[0], [256, 4], [1, 128]]),
                in1=rawap(rft4[:], 128, [rft4[:].ap[0], [256, 4], [1, 128]]),
                op=OP.max)
            nc.vector.tensor_tensor(
                out=pf4[:, grp * 4:grp * 4 + 4, :],
                in0=rawap(tmp4[:], 0, [tmp4[:].ap[0], [128, 4], [2, 64]]),
                in1=rawap(tmp4[:], 1, [tmp4[:].ap[0], [128, 4], [2, 64]]),
                op=OP.max)
            yield
        if debug and img == n_img - 1:
            nc.sync.dma_start(out=dbg["pf4"][:], in_=pf4[:])

        # ---------- conv5 input (2-shift replicate, pad 1) ----------
        c5in = big.tile([128, 66, 68], bf16, tag="cio")
        nc.vector.memset(c5in[:], 0.0)
        for s in range(2):
            c0 = max(0, 1 - s)
            cN = min(68, 65 - s)
            nc.gpsimd.dma_start(
                out=c5in[s * 64:(s + 1) * 64, 1:65, c0:cN],
                in_=pf4[:, :, c0 - 1 + s:cN - 1 + s])
        yield

        # ---------- conv5 (64 -> 128, 3x3, relu) ----------
        c6in = big.tile([128, 68, 72], bf16, tag="c6in")
        nc.vector.memset(c6in[:], 0.0)
        for ci in range(16):
            y = ci * 4
            p5 = psc.tile([128, 256], f32, tag="pc")
            for dy in range(3):
                for dxg in range(2):
                    nc.tensor.matmul(
                        p5[:], w5sb[:, dy, dxg, :],
                        rawap(c5in[:], (y + dy) * 68 + 2 * dxg,
                              [c5in[:].ap[0], [68, 4], [1, 64]]),
                        start=(dy == 0 and dxg == 0),
                        stop=(dy == 2 and dxg == 1))
            nc.scalar.activation(out=c6in[:, 2 + y:2 + y + 4, 2:66],
                                 in_=p5[:], func=AF.Relu, bias=b5sb[:])
            if ci % 2 == 1:
                yield
        if debug and img == n_img - 1:
            nc.sync.dma_start(out=dbg["c6in"][:], in_=c6in[:])

        # ---------- conv6 (128 -> 128, 5x5, pool) ----------
        for ci in range(8):
            y = ci * 8
            p6 = psc.tile([128, 512], f32, tag="pc")
            for dy in range(5):
                for dx in range(5):
                    nc.tensor.matmul(
                        p6[:], w6sb[:, dy, dx, :],
                        rawap(c6in[:], (y + dy) * 72 + dx,
                              [c6in[:].ap[0], [72, 8], [1, 64]]),
                        start=(dy == 0 and dx == 0),
                        stop=(dy == 4 and dx == 4))
            sb6 = cv.tile([128, 8, 64], bf16, tag="cvC")
            nc.scalar.activation(out=sb6[:], in_=p6[:], func=AF.Identity,
                                 bias=b6sb[:])
            t6 = cv.tile([128, 4, 64], bf16, tag="cvD")
            nc.vector.tensor_tensor(
                out=t6[:],
                in0=rawap(sb6[:], 0, [sb6[:].ap[0], [128, 4], [1, 64]]),
                in1=rawap(sb6[:], 64, [sb6[:].ap[0], [128, 4], [1, 64]]),
                op=OP.max)
            hout = rawap(h_sb[:], img * 1024 + ci * 128,
                         [h_sb[:].ap[0], [32, 4], [1, 32]])
            nc.vector.tensor_tensor(
                out=hout,
                in0=rawap(t6[:], 0, [t6[:].ap[0], [64, 4], [2, 32]]),
                in1=rawap(t6[:], 1, [t6[:].ap[0], [64, 4], [2, 32]]),
                op=OP.max)
            yield

    # ---------------- interleaved driver ----------------
    _S = object()

    def interleave(g, c, ratio=3):
        done_g = done_c = False
        while not (done_g and done_c):
            if not done_g:
                for _ in range(ratio):
                    if next(g, _S) is _S:
                        done_g = True
                        break
            if not done_c and next(c, _S) is _S:
                done_c = True

    prev_conv = None
    for img in range(n_img):
        g = glcm_stream(img)
        if prev_conv is None:
            for _ in g:
                pass
        else:
            interleave(g, prev_conv)
        prev_conv = conv_stream(img) if n_ch == 3 else None
    if prev_conv is not None:
        for _ in prev_conv:
            pass

    if n_ch == 3:
        if debug:
            nc.sync.dma_start(out=dbg["h"][:], in_=h_sb[:])
        # ================= fc layers =================
        FW1_BLK = 8
        for kb in range(1024 // FW1_BLK):
            fwt = cv.tile([128, FW1_BLK, 256], bf16, tag="fwt")
            nc.sync.dma_start(
                out=fwt[:],
                in_=rawap(env["fw1p"], kb * FW1_BLK * 128 * 256,
                          [[256, 128], [128 * 256, FW1_BLK], [1, 256]]))
            for j in range(FW1_BLK):
                s = kb * FW1_BLK + j
                nc.tensor.matmul(pfc1[:], h_sb[:, :, s], fwt[:, j, :],
                                 start=(s == 0), stop=(s == 1023))

        h1 = singles.tile([NI, 256], f32, tag="h1")
        nc.vector.tensor_tensor(out=h1[:], in0=pfc1[:], in1=fb1sb[:NI, :],
                                op=OP.add)
        nc.vector.tensor_scalar_max(h1[:], h1[:], 0.0)
        if debug:
            nc.sync.dma_start(out=dbg["h1"][:NI, :], in_=h1[:])

        h1T = singles.tile([128, 2, NI], bf16, tag="h1T")
        for j in range(2):
            ptp = psc.tile([128, NI], f32, tag="pc")
            nc.tensor.matmul(ptp[:], h1[:, j * 128:(j + 1) * 128],
                             id4[:NI, :NI], start=True, stop=True)
            nc.scalar.activation(out=h1T[:, j, :], in_=ptp[:], func=AF.Copy)

        pfc2 = psc.tile([NI, 256], f32, tag="pc")
        for j in range(2):
            nc.tensor.matmul(pfc2[:], h1T[:, j, :], fw2sb[:, j, :],
                             start=(j == 0), stop=(j == 1))
        h2 = singles.tile([NI, 256], f32, tag="h2")
        nc.vector.tensor_tensor(out=h2[:], in0=pfc2[:], in1=fb2sb[:NI, :],
                                op=OP.add)
        nc.vector.tensor_scalar_max(h2[:], h2[:], 0.0)

        h2T = singles.tile([128, 2, NI], bf16, tag="h2T")
        for j in range(2):
            ptp = psc.tile([128, NI], f32, tag="pc")
            nc.tensor.matmul(ptp[:], h2[:, j * 128:(j + 1) * 128],
                             id4[:NI, :NI], start=True, stop=True)
            nc.scalar.activation(out=h2T[:, j, :], in_=ptp[:], func=AF.Copy)

        pfc3 = psc.tile([NI, 1], f32, tag="pc")
        for j in range(2):
            nc.tensor.matmul(pfc3[:], h2T[:, j, :], fw3sb[:, j, :],
                             start=(j == 0), stop=(j == 1))
        osb = singles.tile([NI, 1], f32, tag="osb")
        nc.scalar.activation(out=osb[:], in_=pfc3[:], func=AF.Sigmoid,
                             bias=fb3sb[:NI, :])
        nc.sync.dma_start(out=out4, in_=osb[:])
    else:
        # tiny debug build: just write something to out4
        osb = singles.tile([4, 1], f32, tag="osb")
        nc.vector.memset(osb[:], 0.0)
        nc.sync.dma_start(out=out4, in_=osb[:n_img, :])

    ctx.close()


def kernel(**inputs):
    from concourse.bass_utils import run_bass_kernel_spmd

    inputs = dict(inputs)
    debug = bool(inputs.pop("_debug", False))
    trace = bool(inputs.pop("_trace", False))
    key = ("k", debug)
    if key not in _BUILD_CACHE:
        _BUILD_CACHE[key] = _build(debug=debug)
    nc = _BUILD_CACHE[key]

    packed = _pack_weights(inputs)
    x = np.asarray(inputs["x"], np.float32)
    in_maps = []
    for c in range(N_CORES):
        m = dict(packed)
        m["x4"] = np.ascontiguousarray(x[c * N_IMG:(c + 1) * N_IMG])
        in_maps.append(m)

    res = run_bass_kernel_spmd(nc, in_maps, core_ids=list(range(N_CORES)),
                               trace=trace)
    out = np.concatenate([res.results[c]["out4"] for c in range(N_CORES)],
                         axis=0)
    kernel._last_results = res
    return out



# revision 32
# speedup vs baseline: 2.5537x; 1.0119x over previous
"""Trainium2 Bass kernel for nn_CoOccurrenceMatrixFast.

GLCM (256x256-bin co-occurrence histograms) via one-hot matmuls on the PE
(exact integer counts accumulated in fp32 PSUM), followed by the 6-conv +
3-fc CNN in bf16 with fp32 accumulation.  Pure data parallel: batch 32
sharded as 4 images per NeuronCore across 8 cores.

kernel(**inputs) takes the full unsharded inputs, returns the full [32, 1].
"""

import numpy as np
import ml_dtypes

N_CORES = 8
N_IMG = 4          # images per core
N_CH = 3
NB = 256
MEAN = [0.485, 0.456, 0.406]
STD = [0.229, 0.224, 0.225]
PAD_SENTINEL = 384.0   # never equals -q (q in 0..255); exact in bf16

IMC_R, IMC_C = 258, 260        # conv1 im2col dram buffer [27, 258, 260]
C1O_R, C1O_C = 260, 264        # conv1 output dram buffer [32, 260, 264]

W_SL = 6                       # one-hot window slots
N_SLOT = 513

_BUILD_CACHE = {}


def _pack_weights(inp):
    f32 = np.float32
    bf16 = ml_dtypes.bfloat16
    w1, w2, w3, w4, w5, w6 = (np.asarray(inp[k], f32) for k in
                              ("w1", "w2", "w3", "w4", "w5", "w6"))
    out = {}

    # conv1: lhsT [27, 32];  K row t=(dy*3+dx)*3+ch
    w1p = np.zeros((27, 32), f32)
    for dy in range(3):
        for dx in range(3):
            for ch in range(3):
                w1p[(dy * 3 + dx) * 3 + ch, :] = w1[:, ch, dy, dx]
    out["w1p"] = w1p.astype(bf16)

    # conv2: [8, 2, 128K, 128M]; K=(s*32+ic); M=(oc*4+g)
    w2p = np.zeros((8, 2, 128, 128), f32)
    for r in range(8):
        for g in range(4):
            dy = r - g
            if not (0 <= dy < 5):
                continue
            for s in range(4):
                w2p[r, 0, s * 32:(s + 1) * 32, g::4] = w2[:, :, dy, s].T
            w2p[r, 1, 0:32, g::4] = w2[:, :, dy, 4].T
    out["w2p"] = w2p.astype(bf16)

    # conv3: [3, 128K, 64M]; K=(s*32+ic), s in 0..2
    w3p = np.zeros((3, 128, 64), f32)
    for dy in range(3):
        for s in range(3):
            w3p[dy, s * 32:(s + 1) * 32, :] = w3[:, :, dy, s].T
    out["w3p"] = w3p.astype(bf16)

    # conv4: [6, 3, 128K, 128M]; K=(s*64+ic); M=(oc*2+g)
    w4p = np.zeros((6, 3, 128, 128), f32)
    for r in range(6):
        for g in range(2):
            dy = r - g
            if not (0 <= dy < 5):
                continue
            for s in range(2):
                w4p[r, 0, s * 64:(s + 1) * 64, g::2] = w4[:, :, dy, s].T
                w4p[r, 1, s * 64:(s + 1) * 64, g::2] = w4[:, :, dy, 2 + s].T
            w4p[r, 2, 0:64, g::2] = w4[:, :, dy, 4].T
    out["w4p"] = w4p.astype(bf16)

    # conv5: [3, 2, 128K, 128M]; K=(s*64+ic)
    w5p = np.zeros((3, 2, 128, 128), f32)
    for dy in range(3):
        for s in range(2):
            w5p[dy, 0, s * 64:(s + 1) * 64, :] = w5[:, :, dy, s].T
        w5p[dy, 1, 0:64, :] = w5[:, :, dy, 2].T
    out["w5p"] = w5p.astype(bf16)

    # conv6: [5, 5, 128K(ic), 128M(oc)]
    out["w6p"] = np.ascontiguousarray(w6.transpose(2, 3, 1, 0)).astype(bf16)

    out["b1p"] = np.asarray(inp["b1"], f32).reshape(32, 1)
    out["b2p"] = np.repeat(np.asarray(inp["b2"], f32), 4).reshape(128, 1)
    out["b3p"] = np.asarray(inp["b3"], f32).reshape(64, 1)
    out["b4p"] = np.repeat(np.asarray(inp["b4"], f32), 2).reshape(128, 1)
    out["b5p"] = np.asarray(inp["b5"], f32).reshape(128, 1)
    out["b6p"] = np.asarray(inp["b6"], f32).reshape(128, 1)

    fw1 = np.asarray(inp["fw1"], f32).reshape(256, 128, 1024)     # [o, ch, s]
    out["fw1p"] = np.ascontiguousarray(fw1.transpose(2, 1, 0)).astype(bf16)
    fw2 = np.asarray(inp["fw2"], f32)
    out["fw2p"] = np.ascontiguousarray(fw2.T.reshape(2, 128, 256)).astype(bf16)
    fw3 = np.asarray(inp["fw3"], f32)
    out["fw3p"] = np.ascontiguousarray(fw3.T.reshape(2, 128, 1)).astype(bf16)

    out["fb1bc"] = np.tile(np.asarray(inp["fb1"], f32).reshape(1, 256), (4, 1))
    out["fb2bc"] = np.tile(np.asarray(inp["fb2"], f32).reshape(1, 256), (4, 1))
    out["fb3bc"] = np.tile(np.asarray(inp["fb3"], f32).reshape(1, 1), (4, 1))

    qc = np.zeros((128, 6), f32)
    for c in range(3):
        qc[:, c] = MEAN[c]
        qc[:, 3 + c] = STD[c]
    out["qconst"] = qc

    out["negiota"] = np.tile((-np.arange(256, dtype=f32)).reshape(1, 256),
                             (128, 1)).astype(bf16)
    out["id128"] = np.eye(128, dtype=f32)
    out["id4"] = np.eye(4, dtype=f32)
    return out


def _build(debug=False, n_img=N_IMG, n_ch=N_CH):
    import concourse.bass as bass
    import concourse.tile as tile
    import concourse.mybir as mybir
    from concourse import bacc

    dt = mybir.dt
    f32, bf16 = dt.float32, dt.bfloat16

    nc = bacc.Bacc("TRN2", target_bir_lowering=False, debug=False,
                   num_devices=N_CORES)

    env = {}

    def din(name, shape, dtype):
        ap = nc.dram_tensor(name, shape, dtype, kind="ExternalInput").ap()
        env[name] = ap
        return ap

    din("x4", [n_img, N_CH, 256, 256], f32)
    din("w1p", [27, 32], bf16)
    din("w2p", [8, 2, 128, 128], bf16)
    din("w3p", [3, 128, 64], bf16)
    din("w4p", [6, 3, 128, 128], bf16)
    din("w5p", [3, 2, 128, 128], bf16)
    din("w6p", [5, 5, 128, 128], bf16)
    for nm, p in [("b1p", 32), ("b2p", 128), ("b3p", 64), ("b4p", 128),
                  ("b5p", 128), ("b6p", 128)]:
        din(nm, [p, 1], f32)
    din("fw1p", [1024, 128, 256], bf16)
    din("fw2p", [2, 128, 256], bf16)
    din("fw3p", [2, 128, 1], bf16)
    din("fb1bc", [4, 256], f32)
    din("fb2bc", [4, 256], f32)
    din("fb3bc", [4, 1], f32)
    din("negiota", [128, 256], bf16)
    din("qconst", [128, 6], f32)
    din("id128", [128, 128], f32)
    din("id4", [4, 4], f32)

    env["out4"] = nc.dram_tensor("out4", [n_img, 1], f32,
                                 kind="ExternalOutput").ap()
    dbg = {}
    if debug:
        def dout(name, shape, dtype=bf16):
            dbg[name] = nc.dram_tensor("dbg_" + name, shape, dtype,
                                       kind="ExternalOutput").ap()
        dout("co", [n_img * n_ch, 128, 512])
        dout("pf2", [32, 128, 128])
        dout("pf3", [64, 128, 128])
        dout("pf4", [64, 64, 64])
        dout("c6in", [128, 68, 72])
        dout("h", [128, n_img, 1024])
        dout("h1", [4, 256], f32)
    env["dbg"] = dbg

    with tile.TileContext(nc) as tc:
        _emit(nc, tc, bass, mybir, env, debug, n_img, n_ch)
    nc.compile()
    return nc


def _emit(nc, tc, bass, mybir, env, debug, n_img, n_ch):
    from contextlib import ExitStack
    dt = mybir.dt
    f32, bf16 = dt.float32, dt.bfloat16
    AF = mybir.ActivationFunctionType
    OP = mybir.AluOpType
    x4 = env["x4"]
    out4 = env["out4"]
    dbg = env["dbg"]

    def rawap(base, extra_off, dims):
        return bass.AP(tensor=base.tensor, offset=base.offset + extra_off,
                       ap=dims)

    ctx = ExitStack()
    singles = ctx.enter_context(tc.tile_pool(name="singles", bufs=1))
    dram = ctx.enter_context(tc.tile_pool(name="dram", bufs=1, space="DRAM"))
    psg = ctx.enter_context(tc.tile_pool(name="psg", bufs=1, space="PSUM"))
    psc = ctx.enter_context(tc.tile_pool(name="psc", bufs=2, space="PSUM"))
    psct = ctx.enter_context(tc.tile_pool(name="psct", bufs=2, space="PSUM"))
    gl = ctx.enter_context(tc.tile_pool(name="gl", bufs=2))
    ohp = ctx.enter_context(tc.tile_pool(name="ohp", bufs=2))
    cv = ctx.enter_context(tc.tile_pool(name="cv", bufs=2))
    big = ctx.enter_context(tc.tile_pool(name="big", bufs=1))

    # ---------------- constants to SBUF ----------------
    def load_const(name, shape, dtype, tag=None):
        t = singles.tile(shape, dtype, tag=tag or name)
        nc.sync.dma_start(out=t[:], in_=env[name])
        return t

    def load_w(dname, ntile_shape, dtype):
        # dram [T..., 128K, M] -> sbuf [128K, T..., M]
        sb = singles.tile(ntile_shape, dtype, tag=dname + "sb")
        d = env[dname]
        K = d.ap[-2][1]
        M = d.ap[-1][1]
        nt = 1
        for s, c in d.ap[:-2]:
            nt *= c
        in_dims = [[d.ap[-2][0], K], [K * M, nt], [1, M]]
        nc.sync.dma_start(out=sb[:], in_=rawap(d, 0, in_dims))
        return sb

    w1sb = load_const("w1p", [27, 32], bf16)
    w2sb = load_w("w2p", [128, 8, 2, 128], bf16)
    w3sb = load_w("w3p", [128, 3, 64], bf16)
    w4sb = load_w("w4p", [128, 6, 3, 128], bf16)
    w5sb = load_w("w5p", [128, 3, 2, 128], bf16)
    w6sb = load_w("w6p", [128, 5, 5, 128], bf16)
    fw2sb = load_w("fw2p", [128, 2, 256], bf16)
    fw3sb = load_w("fw3p", [128, 2, 1], bf16)

    b1sb = load_const("b1p", [32, 1], f32)
    b2sb = load_const("b2p", [128, 1], f32)
    b3sb = load_const("b3p", [64, 1], f32)
    b4sb = load_const("b4p", [128, 1], f32)
    b5sb = load_const("b5p", [128, 1], f32)
    b6sb = load_const("b6p", [128, 1], f32)
    fb1sb = load_const("fb1bc", [4, 256], f32)
    fb2sb = load_const("fb2bc", [4, 256], f32)
    fb3sb = load_const("fb3bc", [4, 1], f32)
    negio = load_const("negiota", [128, 256], bf16)
    qcsb = load_const("qconst", [128, 6], f32)
    id128 = load_const("id128", [128, 128], f32)
    id4 = load_const("id4", [4, 4], f32)

    # ---------------- DRAM scratch (zero-init once) ----------------
    imc = [dram.tile([27, IMC_R, IMC_C], bf16, tag=f"imc{i}", name=f"imc{i}")
           for i in range(2)]
    c1o = [dram.tile([32, C1O_R, C1O_C], bf16, tag=f"c1o{i}", name=f"c1o{i}")
           for i in range(2)]

    ZW = 512
    zt = singles.tile([128, ZW], bf16, tag="zeros")
    nc.vector.memset(zt[:], 0.0)
    ZN = 128 * ZW
    for buf, total in [(imc[0][:], 27 * IMC_R * IMC_C),
                       (imc[1][:], 27 * IMC_R * IMC_C),
                       (c1o[0][:], 32 * C1O_R * C1O_C),
                       (c1o[1][:], 32 * C1O_R * C1O_C)]:
        off = 0
        while off < total:
            n = min(ZN, total - off)
            p = n // ZW
            if p >= 1:
                nc.gpsimd.dma_start(
                    out=rawap(buf, off, [[ZW, p], [1, ZW]]),
                    in_=zt[:p, :ZW])
                off += p * ZW
            else:
                nc.gpsimd.dma_start(out=rawap(buf, off, [[n, 1], [1, n]]),
                                    in_=zt[:1, :n])
                off += n

    h_sb = singles.tile([128, n_img, 1024], bf16, tag="h")
    NI = n_img
    pfc1 = psg.tile([NI, 256], f32, tag="pfc1")

    # ================= pipelined streams =================
    # The PE drains its queue strictly in order, so conv(i) stalls idle
    # it even though GLCM(i+1) matmuls are data-ready.  Emitting conv(i)
    # and GLCM(i+1) interleaved keeps the PE stream dense (and the HAM
    # clock-gate warm).
    def glcm_stream(img):
        imcb = imc[img % 2]

        for ch in range(n_ch):
            xbase = x4[img, ch]    # [256, 256] dram ap

            xin = gl.tile([128, 2, 256], f32, tag="xin")
            nc.sync.dma_start(out=xin[:], in_=rawap(
                xbase, 0, [[256, 128], [128 * 256, 2], [1, 256]]))
            xin2 = gl.tile([128, 2, 256], f32, tag="xin2")
            nc.sync.dma_start(out=xin2[:, 0, :], in_=rawap(
                xbase, 256, [[256, 128], [1, 256]]))
            nc.vector.memset(xin2[:, 1, :], 0.0)
            nc.sync.dma_start(out=xin2[:127, 1, :], in_=rawap(
                xbase, 129 * 256, [[256, 127], [1, 256]]))

            # quantize: u = min(relu(std*x+mean), 1)*255; store -floor(u).
            # Each value is stored TWICE ([s,2] pairs) so the one-hot
            # is_equal can read it with a last-dim [1,2] stride-1 AP,
            # qualifying for the DVE 2x perf mode.
            qv = gl.tile([128, 2, 520, 2], bf16, tag="qv")
            qv0 = qv[:]
            nc.vector.memset(qv[:, 1, :, :], PAD_SENTINEL)
            nc.vector.memset(qv[:, 0, 512:513, :], PAD_SENTINEL)
            for (src, dst_off, blk_parts) in (
                    (xin, 0, (128, 128)), (xin2, 520 + 1, (128, 127))):
                r = gl.tile([128, 2, 256], f32, tag="qr")
                nc.scalar.activation(out=r[:], in_=src[:], func=AF.Relu,
                                     bias=qcsb[:, ch:ch + 1],
                                     scale=qcsb[:, 3 + ch:3 + ch + 1])
                u = gl.tile([128, 2, 256], f32, tag="qu")
                nc.vector.tensor_scalar(out=u[:], in0=r[:], scalar1=1.0,
                                        scalar2=float(NB - 1), op0=OP.min,
                                        op1=OP.mult)
                # HW f32->int cast rounds to nearest; shift so that
                # round(u - 0.5 + eps) == floor(u) incl. integer ties.
                u2 = gl.tile([128, 2, 256], f32, tag="qu2")
                nc.vector.tensor_scalar(out=u2[:], in0=u[:], scalar1=0.5,
                                        scalar2=float(2.0 ** -15),
                                        op0=OP.subtract, op1=OP.add)
                qi = gl.tile([128, 2, 256], dt.int32, tag="qi")
                nc.vector.tensor_copy(out=qi[:], in_=u2[:])
                for blk in range(2):
                    np_ = blk_parts[blk]
                    L = dst_off + 256 * blk
                    noff = (L // 520) * 1040 + (L % 520) * 2
                    dst = rawap(qv0, noff,
                                [[qv0.ap[0][0], np_], [2, 256], [1, 2]])
                    qin = rawap(qi[:], 256 * blk,
                                [[qi[:].ap[0][0], np_], [1, 256], [0, 2]])
                    nc.vector.tensor_scalar(out=dst, in0=qin, scalar1=-1.0,
                                            scalar2=None, op0=OP.mult)
            yield

            # G psum: [128, 512] = [G_h | G_v] per left-half
            G = [psg.tile([128, 512], f32, tag=f"G{h}", name=f"G{h}")
                 for h in range(2)]

            n_win = (N_SLOT + W_SL - 1) // W_SL
            ohw = [None] * n_win
            first = [True, True]

            # one-hot [128, slot, blk, value] (dense for the PE); the DVE
            # build reads qv's duplicated pairs with a [1,2] last dim so
            # is_equal qualifies for the 2x perf mode.  One instruction
            # per blk keeps the APs at 3 free dims.
            def build_window(w):
                nslots = min(W_SL, N_SLOT - w * W_SL)
                t = ohp.tile([128, W_SL, 2, 256], bf16, tag="ohw")
                t0 = t[:]
                for b in range(2):
                    out = rawap(t0, b * 256,
                                [t0.ap[0], [512, nslots], [2, 128], [1, 2]])
                    in0 = rawap(qv0, b * 1040 + w * W_SL * 2,
                                [qv0.ap[0], [2, nslots], [0, 128], [1, 2]])
                    in1 = rawap(negio[:], 0,
                                [negio[:].ap[0], [0, nslots], [2, 128],
                                 [1, 2]])
                    nc.vector.tensor_tensor(out=out, in0=in0, in1=in1,
                                            op=OP.is_equal)
                ohw[w] = t

            def chunk_mm(t):
                w1_, s1 = divmod(t, W_SL)
                w2_, s2 = divmod(t + 1, W_SL)
                stop = t == 511
                if t == 255:
                    rhs = ohw[w2_][:, s2, 1, :]
                    for h in range(2):
                        lhsT = ohw[w1_][:, s1, 0, h * 128:(h + 1) * 128]
                        nc.tensor.matmul(G[h][:, 256:512], lhsT, rhs,
                                         start=False, stop=False)
                    return
                rhs = ohw[w2_][:, s2, :, :]
                for h in range(2):
                    lhsT = ohw[w1_][:, s1, 0, h * 128:(h + 1) * 128]
                    nc.tensor.matmul(G[h][:], lhsT, rhs,
                                     start=first[h], stop=stop)
                    first[h] = False

            for w in range(n_win):
                build_window(w)
                lo = max(0, w * W_SL - 1)
                hi = min(512, w * W_SL + W_SL - 1)
                for t in range(lo, hi):
                    chunk_mm(t)
                yield

            # S = G_h + G_v ; co = S + S^T
            s_half = []
            for h in range(2):
                tmp = gl.tile([128, 256], f32, tag="stmp")
                nc.scalar.activation(out=tmp[:], in_=G[h][:, 0:256],
                                     func=AF.Copy)
                s = gl.tile([128, 256], f32, tag=f"s{h}")
                nc.vector.tensor_tensor(out=s[:], in0=G[h][:, 256:512],
                                        in1=tmp[:], op=OP.add)
                s_half.append(s)
            co_t = gl.tile([128, 2, 256], bf16, tag="co")
            for h in range(2):
                tp = psct.tile([128, 256], f32, tag="pt")
                for j in range(2):
                    nc.tensor.matmul(tp[:, j * 128:(j + 1) * 128],
                                     s_half[j][:, h * 128:(h + 1) * 128],
                                     id128[:], start=True, stop=True)
                nc.vector.tensor_tensor(out=co_t[:, h, :], in0=tp[:],
                                        in1=s_half[h][:], op=OP.add)
            if debug:
                nc.sync.dma_start(
                    out=dbg["co"][img * n_ch + ch],
                    in_=co_t[:].rearrange("p a b -> p (a b)"))

            # 9 tap-shifted replicas into the conv1 im2col buffer
            P_IMC = IMC_R * IMC_C
            for dy in range(3):
                for dx in range(3):
                    tpart = (dy * 3 + dx) * 3 + ch
                    off = tpart * P_IMC + (2 - dy) * IMC_C + (2 - dx)
                    nc.gpsimd.dma_start(
                        out=rawap(imcb[:], off,
                                  [[IMC_C, 128], [128 * IMC_C, 2], [1, 256]]),
                        in_=co_t[:])
            yield

    def conv_stream(img):
        imcb = imc[img % 2]
        c1ob = c1o[img % 2]

        # ---------- conv1 (27 -> 32, relu) ----------
        for band in range(16):
            y0 = band * 16
            bt1 = cv.tile([27, 16, 260], bf16, tag="cvA")
            nc.sync.dma_start(out=bt1[:], in_=imcb[:, y0 + 1:y0 + 17, :])
            st1 = cv.tile([32, 16, 256], bf16, tag="cvB")
            for ci in range(8):
                p1 = psc.tile([32, 512], f32, tag="pc")
                nc.tensor.matmul(p1[:], w1sb[:],
                                 bt1[:, 2 * ci:2 * ci + 2, 1:257],
                                 start=True, stop=True)
                nc.scalar.activation(out=st1[:, 2 * ci:2 * ci + 2, :],
                                     in_=p1[:], func=AF.Relu, bias=b1sb[:])
            nc.gpsimd.dma_start(out=c1ob[:, 2 + y0:2 + y0 + 16, 2:258],
                                in_=st1[:])
            yield
        # ---------- conv2 (32 -> 32, 5x5, pool) ----------
        # c3in border zeros up front (interior is overwritten by the
        # per-band shift DMAs below; only cells conv3 reads but the
        # shifts never write need zeroing).
        c3in = big.tile([128, 130, 132], bf16, tag="cio")
        nc.vector.memset(c3in[:, 0:1, :], 0.0)
        nc.vector.memset(c3in[:, 129:130, :], 0.0)
        nc.vector.memset(c3in[0:32, 1:129, 0:1], 0.0)
        nc.vector.memset(c3in[64:96, 1:129, 127:128], 0.0)
        nc.vector.memset(c3in[96:128, 1:129, 126:128], 0.0)

        pf2 = big.tile([32, 128, 128], bf16, tag="pf")
        for band in range(32):
            y0 = band * 8
            bt2 = cv.tile([128, 12, 260], bf16, tag="cvA")
            for s in range(4):
                nc.sync.dma_start(out=bt2[s * 32:(s + 1) * 32, :, :],
                                  in_=rawap(
                    c1ob[:], y0 * C1O_C + s,
                    [[C1O_R * C1O_C, 32], [C1O_C, 12], [1, 260]]))
            rft2 = cv.tile([32, 8, 256], bf16, tag="cvB")
            for ci in range(2):
                p2 = psc.tile([128, 256], f32, tag="pc")
                for rr in range(8):
                    for dxg in range(2):
                        nc.tensor.matmul(
                            p2[:], w2sb[:, rr, dxg, :],
                            bt2[:, 4 * ci + rr, 4 * dxg:4 * dxg + 256],
                            start=(rr == 0 and dxg == 0),
                            stop=(rr == 7 and dxg == 1))
                bsb2 = cv.tile([128, 256], bf16, tag="cvC")
                nc.scalar.activation(out=bsb2[:], in_=p2[:],
                                     func=AF.Identity, bias=b2sb[:])
                nc.gpsimd.dma_start(
                    out=rawap(rft2[:], ci * 1024,
                              [rft2[:].ap[0], [256, 4], [1, 256]]),
                    in_=bsb2[:])
            tmp2 = cv.tile([32, 4, 256], bf16, tag="cvD")
            nc.vector.tensor_tensor(
                out=tmp2[:],
                in0=rawap(rft2[:], 0, [rft2[:].ap[0], [512, 4], [1, 256]]),
                in1=rawap(rft2[:], 256, [rft2[:].ap[0], [512, 4], [1, 256]]),
                op=OP.max)
            nc.vector.tensor_tensor(
                out=pf2[:, band * 4:band * 4 + 4, :],
                in0=rawap(tmp2[:], 0, [tmp2[:].ap[0], [256, 4], [2, 128]]),
                in1=rawap(tmp2[:], 1, [tmp2[:].ap[0], [256, 4], [2, 128]]),
                op=OP.max)
            # incremental conv3-input shifts: this band's 4 pf2 rows go
            # straight into c3in so conv3 can start the moment conv2 ends
            r0 = band * 4
            for s in range(4):
                c0 = max(0, 1 - s)
                cN = min(132, 129 - s)
                nc.scalar.dma_start(
                    out=c3in[s * 32:(s + 1) * 32, 1 + r0:1 + r0 + 4, c0:cN],
                    in_=pf2[:, r0:r0 + 4, c0 - 1 + s:cN - 1 + s])
            yield
        if debug and img == n_img - 1:
            nc.sync.dma_start(out=dbg["pf2"][:], in_=pf2[:])

        # ---------- conv3 (32 -> 64, 3x3, relu) ----------
        pf3 = big.tile([64, 128, 128], bf16, tag="pf")
        for ci in range(32):
            y = ci * 4
            p3 = psc.tile([64, 512], f32, tag="pc")
            for dy in range(3):
                nc.tensor.matmul(p3[:], w3sb[:, dy, :],
                                 c3in[:, y + dy:y + dy + 4, 0:128],
                                 start=(dy == 0), stop=(dy == 2))
            nc.scalar.activation(out=pf3[:, y:y + 4, :], in_=p3[:],
                                 func=AF.Relu, bias=b3sb[:])
            if ci % 4 == 3:
                yield
        if debug and img == n_img - 1:
            nc.sync.dma_start(out=dbg["pf3"][:], in_=pf3[:])

        # ---------- conv4 input (2-shift replicate, pad 2) ----------
        # border-only zeroing; the two interior shifts ride separate DMA
        # queues so they overlap instead of serializing on gpsimd
        c4in = big.tile([128, 132, 132], bf16, tag="cio")
        nc.vector.memset(c4in[:, 0:2, :], 0.0)
        nc.vector.memset(c4in[:, 130:132, :], 0.0)
        nc.vector.memset(c4in[:, 2:130, 0:2], 0.0)
        nc.vector.memset(c4in[:, 2:130, 129:132], 0.0)
        for s in range(2):
            c0 = max(0, 2 - s)
            cN = min(132, 130 - s)
            q = nc.sync if s == 0 else nc.scalar
            q.dma_start(
                out=c4in[s * 64:(s + 1) * 64, 2:130, c0:cN],
                in_=pf3[:, :, c0 - 2 + s:cN - 2 + s])
        yield

        # ---------- conv4 (64 -> 64, 5x5, pool) ----------
        pf4 = big.tile([64, 64, 64], bf16, tag="pf")
        for grp in range(16):     # 16 groups of 4 chunks (2 rows each)
            rft4 = cv.tile([64, 8, 128], bf16, tag="cvB")
            for cj in range(4):
                ci = grp * 4 + cj
                y = ci * 2
                p4 = psc.tile([128, 128], f32, tag="pc")
                for rr in range(6):
                    for dxg in range(3):
                        nc.tensor.matmul(
                            p4[:], w4sb[:, rr, dxg, :],
                            c4in[:, y + rr, 2 * dxg:2 * dxg + 128],
                            start=(rr == 0 and dxg == 0),
                            stop=(rr == 5 and dxg == 2))
                bsb4 = cv.tile([128, 128], bf16, tag="cvC")
                nc.scalar.activation(out=bsb4[:], in_=p4[:],
                                     func=AF.Identity, bias=b4sb[:])
                nc.gpsimd.dma_start(
                    out=rawap(rft4[:], cj * 256,
                              [rft4[:].ap[0], [128, 2], [1, 128]]),
                    in_=bsb4[:])
            tmp4 = cv.tile([64, 4, 128], bf16, tag="cvD")
            nc.vector.tensor_tensor(
                out=tmp4[:],
                in0=rawap(rft4[:], 0, [rft4[:].ap[0], [256, 4], [1, 128]]),
                in1=rawap(rft4[:], 128, [rft4[:].ap[0], [256, 4], [1, 128]]),
                op=OP.max)
            nc.vector.tensor_tensor(
                out=pf4[:, grp * 4:grp * 4 + 4, :],
                in0=rawap(tmp4[:], 0, [tmp4[:].ap[0], [128, 4], [2, 64]]),
                in1=rawap(tmp4[:], 1, [tmp4[:].ap[0], [128, 4], [2, 64]]),
                op=OP.max)
            yield
        if debug and img == n_img - 1:
            nc.sync.dma_start(out=dbg["pf4"][:], in_=pf4[:])

        # ---------- conv5 input (2-shift replicate, pad 1) ----------
        c5in = big.tile([128, 66, 68], bf16, tag="cio")
        nc.vector.memset(c5in[:, 0:1, :], 0.0)
        nc.vector.memset(c5in[:, 65:66, :], 0.0)
        nc.vector.memset(c5in[0:64, 1:65, 0:1], 0.0)
        nc.vector.memset(c5in[:, 1:65, 64:66], 0.0)
        for s in range(2):
            c0 = max(0, 1 - s)
            cN = min(68, 65 - s)
            q = nc.sync if s == 0 else nc.scalar
            q.dma_start(
                out=c5in[s * 64:(s + 1) * 64, 1:65, c0:cN],
                in_=pf4[:, :, c0 - 1 + s:cN - 1 + s])
        yield

        # ---------- conv5 (64 -> 128, 3x3, relu) ----------
        c6in = big.tile([128, 68, 72], bf16, tag="c6in")
        nc.vector.memset(c6in[:, 0:2, :], 0.0)
        nc.vector.memset(c6in[:, 66:68, :], 0.0)
        nc.vector.memset(c6in[:, 2:66, 0:2], 0.0)
        nc.vector.memset(c6in[:, 2:66, 66:68], 0.0)
        for ci in range(16):
            y = ci * 4
            p5 = psc.tile([128, 256], f32, tag="pc")
            for dy in range(3):
                for dxg in range(2):
                    nc.tensor.matmul(
                        p5[:], w5sb[:, dy, dxg, :],
                        rawap(c5in[:], (y + dy) * 68 + 2 * dxg,
                              [c5in[:].ap[0], [68, 4], [1, 64]]),
                        start=(dy == 0 and dxg == 0),
                        stop=(dy == 2 and dxg == 1))
            nc.scalar.activation(out=c6in[:, 2 + y:2 + y + 4, 2:66],
                                 in_=p5[:], func=AF.Relu, bias=b5sb[:])
            if ci % 2 == 1:
                yield
        if debug and img == n_img - 1:
            nc.sync.dma_start(out=dbg["c6in"][:], in_=c6in[:])

        # ---------- conv6 (128 -> 128, 5x5, pool) ----------
        for ci in range(8):
            y = ci * 8
            p6 = psc.tile([128, 512], f32, tag="pc")
            for dy in range(5):
                for dx in range(5):
                    nc.tensor.matmul(
                        p6[:], w6sb[:, dy, dx, :],
                        rawap(c6in[:], (y + dy) * 72 + dx,
                              [c6in[:].ap[0], [72, 8], [1, 64]]),
                        start=(dy == 0 and dx == 0),
                        stop=(dy == 4 and dx == 4))
            sb6 = cv.tile([128, 8, 64], bf16, tag="cvC")
            nc.scalar.activation(out=sb6[:], in_=p6[:], func=AF.Identity,
                                 bias=b6sb[:])
            t6 = cv.tile([128, 4, 64], bf16, tag="cvD")
            nc.vector.tensor_tensor(
                out=t6[:],
                in0=rawap(sb6[:], 0, [sb6[:].ap[0], [128, 4], [1, 64]]),
                in1=rawap(sb6[:], 64, [sb6[:].ap[0], [128, 4], [1, 64]]),
                op=OP.max)
            hout = rawap(h_sb[:], img * 1024 + ci * 128,
                         [h_sb[:].ap[0], [32, 4], [1, 32]])
            nc.vector.tensor_tensor(
                out=hout,
                in0=rawap(t6[:], 0, [t6[:].ap[0], [64, 4], [2, 32]]),
                in1=rawap(t6[:], 1, [t6[:].ap[0], [64, 4], [2, 32]]),
                op=OP.max)
            yield

    # ---------------- interleaved driver ----------------
    _S = object()

    def interleave(g, c, ratio=3):
        done_g = done_c = False
        while not (done_g and done_c):
            if not done_g:
                for _ in range(ratio):
                    if next(g, _S) is _S:
                        done_g = True
                        break
            if not done_c and next(c, _S) is _S:
                done_c = True

    prev_conv = None
    for img in range(n_img):
        g = glcm_stream(img)
        if prev_conv is None:
            for _ in g:
                pass
        else:
            interleave(g, prev_conv)
        prev_conv = conv_stream(img) if n_ch == 3 else None
    if prev_conv is not None:
        for _ in prev_conv:
            pass

    if n_ch == 3:
        if debug:
            nc.sync.dma_start(out=dbg["h"][:], in_=h_sb[:])
        # ================= fc layers =================
        FW1_BLK = 8
        for kb in range(1024 // FW1_BLK):
            fwt = cv.tile([128, FW1_BLK, 256], bf16, tag="fwt")
            nc.sync.dma_start(
                out=fwt[:],
                in_=rawap(env["fw1p"], kb * FW1_BLK * 128 * 256,
                          [[256, 128], [128 * 256, FW1_BLK], [1, 256]]))
            for j in range(FW1_BLK):
                s = kb * FW1_BLK + j
                nc.tensor.matmul(pfc1[:], h_sb[:, :, s], fwt[:, j, :],
                                 start=(s == 0), stop=(s == 1023))

        h1 = singles.tile([NI, 256], f32, tag="h1")
        nc.vector.tensor_tensor(out=h1[:], in0=pfc1[:], in1=fb1sb[:NI, :],
                                op=OP.add)
        nc.vector.tensor_scalar_max(h1[:], h1[:], 0.0)
        if debug:
            nc.sync.dma_start(out=dbg["h1"][:NI, :], in_=h1[:])

        h1T = singles.tile([128, 2, NI], bf16, tag="h1T")
        for j in range(2):
            ptp = psc.tile([128, NI], f32, tag="pc")
            nc.tensor.matmul(ptp[:], h1[:, j * 128:(j + 1) * 128],
                             id4[:NI, :NI], start=True, stop=True)
            nc.scalar.activation(out=h1T[:, j, :], in_=ptp[:], func=AF.Copy)

        pfc2 = psc.tile([NI, 256], f32, tag="pc")
        for j in range(2):
            nc.tensor.matmul(pfc2[:], h1T[:, j, :], fw2sb[:, j, :],
                             start=(j == 0), stop=(j == 1))
        h2 = singles.tile([NI, 256], f32, tag="h2")
        nc.vector.tensor_tensor(out=h2[:], in0=pfc2[:], in1=fb2sb[:NI, :],
                                op=OP.add)
        nc.vector.tensor_scalar_max(h2[:], h2[:], 0.0)

        h2T = singles.tile([128, 2, NI], bf16, tag="h2T")
        for j in range(2):
            ptp = psc.tile([128, NI], f32, tag="pc")
            nc.tensor.matmul(ptp[:], h2[:, j * 128:(j + 1) * 128],
                             id4[:NI, :NI], start=True, stop=True)
            nc.scalar.activation(out=h2T[:, j, :], in_=ptp[:], func=AF.Copy)

        pfc3 = psc.tile([NI, 1], f32, tag="pc")
        for j in range(2):
            nc.tensor.matmul(pfc3[:], h2T[:, j, :], fw3sb[:, j, :],
                             start=(j == 0), stop=(j == 1))
        osb = singles.tile([NI, 1], f32, tag="osb")
        nc.scalar.activation(out=osb[:], in_=pfc3[:], func=AF.Sigmoid,
                             bias=fb3sb[:NI, :])
        nc.sync.dma_start(out=out4, in_=osb[:])
    else:
        # tiny debug build: just write something to out4
        osb = singles.tile([4, 1], f32, tag="osb")
        nc.vector.memset(osb[:], 0.0)
        nc.sync.dma_start(out=out4, in_=osb[:n_img, :])

    ctx.close()


def kernel(**inputs):
    from concourse.bass_utils import run_bass_kernel_spmd

    inputs = dict(inputs)
    debug = bool(inputs.pop("_debug", False))
    trace = bool(inputs.pop("_trace", False))
    key = ("k", debug)
    if key not in _BUILD_CACHE:
        _BUILD_CACHE[key] = _build(debug=debug)
    nc = _BUILD_CACHE[key]

    packed = _pack_weights(inputs)
    x = np.asarray(inputs["x"], np.float32)
    in_maps = []
    for c in range(N_CORES):
        m = dict(packed)
        m["x4"] = np.ascontiguousarray(x[c * N_IMG:(c + 1) * N_IMG])
        in_maps.append(m)

    res = run_bass_kernel_spmd(nc, in_maps, core_ids=list(range(N_CORES)),
                               trace=trace)
    out = np.concatenate([res.results[c]["out4"] for c in range(N_CORES)],
                         axis=0)
    kernel._last_results = res
    return out

